# revision 23
# baseline (speedup 1.0000x reference)
"""Trainium2 Bass kernel for nn_Dep_Context_80109730005366.

Math notes (exact restructurings of the reference):
  - ctx = (q @ key) @ value is reassociated as q @ (key @ value); KV is
    [hid, c] so the huge [hw, hw] energy matrix never materializes.
  - The 1x1 conv (proj_W) and the BN scale commute with the bilinear
    upsample, so we contract KV with proj_W into a per-part [hid, hid]
    matrix (KVW) and upsample 10 channels instead of 256.
  - Coord features are input-independent; everything derived from them
    (cf, key/query constant terms) is precomputed on host as tiny matrices.

Sharding: 8 cores = 4 batches x 2 half-part groups. Core k handles batch
n = k//2 and parts {0,1,2} (k even) or {3,4,5} (k odd). The dominant HBM
read (p_fea) is SPLIT across the pair: each core reads only its h-half of
p_fea[n], computes partial KVW-for-all-6-parts over that half-space, and a
tiny [60,10] pair ReduceScatter (CCE add) yields each core the full-space
KVW for exactly its own 3 parts. Per-part work (hu maxpool, q, ctx,
upsample, BN, output) stays part-split over the full space.
"""

import numpy as np

import bass_rust
import concourse.bass as bass
import concourse.tile as tile
from concourse import mybir
from concourse.bass_utils import run_bass_kernel_spmd
from concourse.vector_clock import ScopedClock

EPS = 1e-5
N, C, H, W = 4, 256, 96, 96
HP, WP = 48, 48
HID, PARTS = 10, 6
X = HP * WP        # 2304
XH = X // 2        # 1152 (x-columns in one h-half)
PPC = 3            # parts per core
PL = PPC * HID     # planes per core = 30
NCB = 748          # cbank columns
F32 = mybir.dt.float32

# ---------------------------------------------------------------------------
# Workaround: this container's walrus codegen rejects instructions carrying
# more than a couple of semaphore waits ("Too many sync wait commands").
# TileContext's exit path puts every outstanding wait on one Drain; spread
# them over a chain of single-wait nops instead.
# ---------------------------------------------------------------------------
_MAX_WAITS = 1


def _patched_drain_and_barrier(self, tick_clock, wait_clock):
    nc = self.nc
    drain_inst = nc.sync.drain()
    wait_clock.add_sem_waits(
        drain_inst.ins, ScopedClock({None: tick_clock.global_clock})
    )
    si = drain_inst.ins.sync_info
    waits = list(si.on_wait) if si is not None else []
    updates = list(si.on_update) if si is not None else []
    if len(waits) > _MAX_WAITS:
        drain_inst.ins.sync_info = bass_rust.SyncInfo(
            on_wait=waits[:_MAX_WAITS], on_update=updates
        )
        rest = waits[_MAX_WAITS:]
        for i in range(0, len(rest), _MAX_WAITS):
            nop = nc.sync.nop(nofuse=True, hint="split_drain_wait")
            nop.ins.sync_info = bass_rust.SyncInfo(
                on_wait=rest[i : i + _MAX_WAITS], on_update=[]
            )
    nc.all_engine_barrier()
    assert self.sems is not None
    popped = nc._tile_sem_poison_stack.pop()
    assert popped is self._sem_poison
    nc.clear_and_free_semaphores(list(self.sems.allocated().values()))
    nc.all_engine_barrier()


tile.TileContext._drain_and_barrier = _patched_drain_and_barrier

_BODY_MAX_WAITS = 1


def _split_excess_waits(nc, maxw=_BODY_MAX_WAITS):
    """Post-pass: any instruction carrying more than `maxw` semaphore waits
    gets the excess hoisted onto same-engine nops inserted right before it
    (the engine sequencer blocks on those first, preserving semantics)."""
    eng_map = {
        mybir.EngineType.SP: nc.sync,
        mybir.EngineType.PE: nc.tensor,
        mybir.EngineType.DVE: nc.vector,
        mybir.EngineType.Activation: nc.scalar,
        mybir.EngineType.Pool: nc.gpsimd,
    }

    def make_nop(engine_type, waits):
        bi = eng_map[engine_type].nop(nofuse=True, hint="wait_split")
        # pop it off the tail of the current bb; we'll splice it manually
        cur = nc.cur_bb.bb
        lst = cur.instructions
        assert lst[-1].name == bi.ins.name
        cur.instructions = lst[:-1]
        bi.ins.sync_info = bass_rust.SyncInfo(on_wait=waits, on_update=[])
        return bi.ins

    for bb in nc.m.functions[0].blocks:
        insts = bb.instructions
        out = []
        changed = False
        for inst in insts:
            si = inst.sync_info
            waits = list(si.on_wait) if si is not None else []
            if len(waits) > maxw:
                updates = list(si.on_update) if si is not None else []
                extra, keep = waits[:-maxw], waits[-maxw:]
                for j in range(0, len(extra), maxw):
                    out.append(make_nop(inst.engine, extra[j : j + maxw]))
                inst.sync_info = bass_rust.SyncInfo(on_wait=keep, on_update=updates)
                changed = True
            out.append(inst)
        if changed:
            bb.instructions = out


# ---------------------------------------------------------------------------
# Host-side constant precomputation (all tiny; heavy tensors stay on device)
# ---------------------------------------------------------------------------
def _coord_feats(hp, wp):
    xs = np.arange(wp, dtype=np.float32)
    ys = np.arange(hp, dtype=np.float32)
    xmin = xs / wp * 2 - 1
    xmax = (xs + 1) / wp * 2 - 1
    xctr = (xmin + xmax) / 2
    ymin = ys / hp * 2 - 1
    ymax = (ys + 1) / hp * 2 - 1
    yctr = (ymin + ymax) / 2
    Xb = lambda v: np.broadcast_to(v[None, :], (hp, wp))
    Yb = lambda v: np.broadcast_to(v[:, None], (hp, wp))
    ones = np.ones((hp, wp), np.float32)
    return np.stack(
        [Xb(xmin), Yb(ymin), Xb(xmax), Yb(ymax), Xb(xctr), Yb(yctr),
         ones / wp, ones / hp], axis=0,
    ).astype(np.float32)


def _interp_matrix(out_n, in_n):
    pos = np.arange(out_n, dtype=np.float32) * ((in_n - 1) / (out_n - 1))
    i0 = np.clip(np.floor(pos).astype(np.int64), 0, in_n - 1)
    i1 = np.clip(i0 + 1, 0, in_n - 1)
    w1 = (pos - i0).astype(np.float32)
    M = np.zeros((out_n, in_n), np.float32)
    for r in range(out_n):
        M[r, i0[r]] += 1 - w1[r]
        M[r, i1[r]] += w1[r]
    return M


# ---------------------------------------------------------------------------
# Device program (built once, shared SPMD across all 8 cores)
# cbank column map (all rows fp32):
#   0:70    stat0   [128] key_W.T | WpS.T for channels 0:128 (6 parts)
#   70:140  stat1   [128] same, channels 128:256
#   140:236 id96    [96]
#   236:332 mwT     [48]  Mw.T
#   332:428 mhT     [48]  Mh.T
#   428:498 id70    [70]
#   498:528 qhu4    [128] query_W[:, :10].T block-diag, replicated at
#                         partition offsets 32g for the 4 hu row-groups
#   528:558 bnb     [96]  per-plane BN bias (this core's parts)
#   558:628 statcf  [9]   key coord-const K9.T | zeros
#   628:658 qcf     [9]   query coord-const Q9.T tiled x3
#   658:688 mask3   [30]  block-diag ones mask
#   688:718 selM    [30]  block-diag identity
# ---------------------------------------------------------------------------
def _build_program(reps=1, barrier=False, use_collective=True, upto="full"):
    nc = bass.Bass()
    dt = F32
    STAGES = ["pool", "stat", "kvw", "q", "ctx", "ups1", "full"]
    lvl = STAGES.index(upto)

    pfe = nc.dram_tensor("pfe", [128, 2 * XH * 4], dt, kind="ExternalInput")
    hu4 = nc.dram_tensor("hu4", [64, 4608], dt, kind="ExternalInput")
    cbank = nc.dram_tensor("cbank", [128, NCB], dt, kind="ExternalInput")
    cf9c = nc.dram_tensor("cf9c", [9, X + XH], dt, kind="ExternalInput")
    out3 = nc.dram_tensor("out3", [H, PL * W], dt, kind="ExternalOutput")

    def alt_copy(idx, out, in_):
        # alternate psum->sbuf copies between ACT and DVE to balance engines
        if idx % 2 == 0:
            nc.scalar.copy(out, in_)
        else:
            nc.vector.tensor_copy(out, in_)

    with tile.TileContext(nc) as tc:
      for _rep in range(reps):
        with (
            tc.tile_pool(name="consts", bufs=1) as consts,
            tc.tile_pool(name="pfe_in", bufs=4) as pfe_pool,
            tc.tile_pool(name="p1", bufs=2) as p1_pool,
            tc.tile_pool(name="pf", bufs=1) as pf_pool,
            tc.tile_pool(name="hu", bufs=1) as hu_pool,
            tc.tile_pool(name="big", bufs=1) as big,
            tc.tile_pool(name="small", bufs=2) as small,
            tc.tile_pool(name="dram", bufs=1, space="DRAM") as dram,
            tc.tile_pool(name="psA", bufs=2, space="PSUM") as psA,
            tc.tile_pool(name="psS", bufs=2, space="PSUM") as psS,
            tc.tile_pool(name="psCK", bufs=2, space="PSUM") as psCK,
            tc.tile_pool(name="psU", bufs=2, space="PSUM") as psU,
        ):
            # ---- constants ----------------------------------------------
            cb = consts.tile([128, NCB], dt, tag="cbank", name="cbank")
            nc.sync.dma_start(cb[:], cbank[:])
            cf9_sb = consts.tile([9, X + XH], dt, tag="cf9", name="cf9")
            nc.sync.dma_start(cf9_sb[:], cf9c[:])
            stat0 = cb[0:128, 0:70]
            stat1 = cb[0:128, 70:140]
            id96 = cb[0:96, 140:236]
            mwT = cb[0:WP, 236:332]
            mhT = cb[0:HP, 332:428]
            id70 = cb[0:70, 428:498]
            qhu_g = [cb[0:64, 498:528], cb[0:64, 718:748]]
            bnb = cb[0:H, 528:558]
            statcf = cb[0:9, 558:628]
            qcf = cb[0:9, 628:658]
            mask3 = cb[0:PL, 658:688]
            selM = cb[0:PL, 688:718]
            cf9 = cf9_sb[:, 0:X]
            cf9h = cf9_sb[:, X : X + XH]

            # ---- input DMAs ---------------------------------------------
            hu_sb = hu_pool.tile([64, 48, 96], dt, tag="hu_sb")
            nc.scalar.dma_start(
                hu_sb[:], hu4.rearrange("p (r w) -> p r w", w=96)
            )

            pf_t = [
                pf_pool.tile([128, 24, 48], dt, tag=f"pf{t}", name=f"pf{t}")
                for t in range(2)
            ]
            chunks = []
            for c_i in range(8):
                hh4, t = c_i // 2, c_i % 2
                ch = pfe_pool.tile([128, 12, 96], dt, tag="chunk", name="chunk")
                nc.sync.dma_start(
                    ch[:],
                    pfe[:, t * 4608 + hh4 * 1152 : t * 4608 + (hh4 + 1) * 1152]
                    .rearrange("c (r w) -> c r w", w=96),
                )
                chunks.append((hh4, t, ch))

            # ---- p_fea half maxpool (DVE stage1, Pool stage2) -----------
            for hh4, t, ch in chunks:
                p1 = p1_pool.tile([128, 12, 48], dt, tag="p1", name="p1")
                ch4 = ch.rearrange("c r (w2 two) -> c r w2 two", two=2)
                nc.vector.tensor_max(p1[:], ch4[:, :, :, 0], ch4[:, :, :, 1])
                p14 = p1.rearrange("c (h2 two) w -> c h2 two w", two=2)
                nc.vector.tensor_max(
                    pf_t[t][:, hh4 * 6 : (hh4 + 1) * 6, :],
                    p14[:, :, 0, :],
                    p14[:, :, 1, :],
                )

            # ---- hu maxpool ---------------------------------------------
            hu1 = hu_pool.tile([64, 48, 48], dt, tag="hu1")
            hv = hu_sb.rearrange("p r (w2 two) -> p r w2 two", two=2)
            nc.vector.tensor_max(hu1[:], hv[:, :, :, 0], hv[:, :, :, 1])
            hup = hu_pool.tile([64, 24, 48], dt, tag="hup")
            h1v = hu1.rearrange("p (h2 two) w -> p h2 two w", two=2)
            nc.vector.tensor_max(hup[:], h1v[:, :, 0, :], h1v[:, :, 1, :])
            hup_f = hup.rearrange("p h w -> p (h w)")
            zeros_sb = small.tile([H, W], dt, tag="zeros", name="zeros", bufs=1)
            nc.gpsimd.memset(zeros_sb[:], 0.0)

            # ---- debug early-exit: zero-fill output and stop ------------
            out_sb0 = None
            if lvl < 6:
                out_sb0 = big.tile([H, PL * W], dt, tag="out_sb")
                nc.gpsimd.memset(out_sb0[:], 0.0)

            def _dbg_finish():
                nc.sync.dma_start(out3[:], out_sb0[:])

            if lvl < 1:
                _dbg_finish()
                continue

            # ---- key|WpPf stat matmul over this half's pf ---------------
            keywp = big.tile([70, XH], dt, tag="keywp")
            pf_f = [t_.rearrange("c h w -> c (h w)") for t_ in pf_t]
            for xi in range(4):
                x0 = xi * 288
                ps = psA.tile([70, 288], dt, tag="psA", name="ps")
                nc.tensor.matmul(
                    ps[:], stat0, pf_f[0][:, x0 : x0 + 288],
                    start=True, stop=False,
                )
                nc.tensor.matmul(
                    ps[:], stat1, pf_f[1][:, x0 : x0 + 288],
                    start=False, stop=False,
                )
                nc.tensor.matmul(
                    ps[:], statcf, cf9h[:, x0 : x0 + 288],
                    start=False, stop=True,
                )
                alt_copy(xi, keywp[:, x0 : x0 + 288], ps[:])

            if lvl < 2:
                _dbg_finish()
                continue

            # ---- transpose -> partial KVW^T [60, 10] --------------------
            kwT = big.tile([128, 9, 70], dt, tag="kwT")
            for b in range(9):
                tp = psS.tile([128, 70], dt, tag="pss", name="tp")
                nc.tensor.transpose(
                    tp[:], keywp[:, b * 128 : (b + 1) * 128], id70
                )
                alt_copy(b, kwT[:, b, :], tp[:])
            kvwT_ps = psCK.tile([60, 10], dt, tag="ck", name="kvwT_ps")
            for b in range(9):
                nc.tensor.matmul(
                    kvwT_ps[:], kwT[:, b, 10:70], kwT[:, b, 0:10],
                    start=(b == 0), stop=(b == 8),
                )
            kvwT_sb = small.tile([60, 10], dt, tag="kvwT_sb")
            nc.vector.tensor_copy(kvwT_sb[:], kvwT_ps[:])

            # ---- pair ReduceScatter: full-space KVW^T for my 3 parts ----
            kvw_in = dram.tile([60, 10], dt, tag="kvw_in", name="kvw_in")
            kvw_out = dram.tile([30, 10], dt, tag="kvw_out", name="kvw_out")
            nc.scalar.dma_start(kvw_in[:], kvwT_sb[:])
            if use_collective:
                nc.gpsimd.collective_compute(
                    "ReduceScatter", mybir.AluOpType.add,
                    replica_groups=[[0, 1], [2, 3], [4, 5], [6, 7]],
                    ins=[kvw_in.opt()], outs=[kvw_out.opt()],
                )
            else:
                nc.gpsimd.dma_start(kvw_out[:], kvw_in[0:30, :])
            kvwT_my = small.tile([30, 10], dt, tag="kvwT_my")
            nc.sync.dma_start(kvwT_my[:], kvw_out[:])

            if lvl < 3:
                _dbg_finish()
                continue

            # ---- q matmuls (PE stays busy while the collective flies) ---
            q_sb = big.tile([PL, X], dt, tag="q_sb")
            for c_ in range(8):
                g, hx = c_ // 4, (c_ % 4) * 288
                x0 = g * 1152 + hx
                ps = psA.tile([PL, 288], dt, tag="psA", name="ps")
                nc.tensor.matmul(
                    ps[:], qhu_g[g], hup_f[:, hx : hx + 288],
                    start=True, stop=False,
                )
                nc.tensor.matmul(
                    ps[:], qcf, cf9[:, x0 : x0 + 288], start=False, stop=True
                )
                alt_copy(c_ + 1, q_sb[:, x0 : x0 + 288], ps[:])

            if lvl < 4:
                _dbg_finish()
                continue

            # ---- block-diag KVW via mask-mul + matmul -------------------
            S_sb = small.tile([PL, PL], dt, tag="S_sb")
            for j in range(3):
                nc.vector.tensor_mul(
                    S_sb[:, j * 10 : (j + 1) * 10], kvwT_my[:],
                    mask3[:, j * 10 : (j + 1) * 10],
                )
            kvwbd_ps = psCK.tile([PL, PL], dt, tag="ck", name="kvwbd_ps")
            nc.tensor.matmul(kvwbd_ps[:], S_sb[:], selM)
            kvwbd = small.tile([PL, PL], dt, tag="kvwbd")
            nc.scalar.copy(kvwbd[:], kvwbd_ps[:])

            # ---- ctx, transposed, plane-major free layout ---------------
            q3 = q_sb.rearrange("p (h w) -> p h w", h=HP)
            ctxT = big.tile([WP, PL * HP], dt, tag="ctxT")
            ctxT_hi = ctxT.rearrange("w (i h) -> w h i", i=PL)
            for g3 in range(3):
                cps = psCK.tile([WP, 16 * PL], dt, tag="ck", name="cps")
                for hh in range(16):
                    hp_i = g3 * 16 + hh
                    nc.tensor.matmul(
                        cps[:, hh * PL : (hh + 1) * PL],
                        q3[:, hp_i, :],
                        kvwbd[:],
                    )
                cps_v = cps.rearrange("w (h i) -> w h i", h=16)
                alt_copy(g3, ctxT_hi[:, g3 * 16 : (g3 + 1) * 16, :], cps_v[:])

            if lvl < 5:
                _dbg_finish()
                continue

            # ---- upsample stage 1: contract w' --------------------------
            a_sb = big.tile([W, PL * HP], dt, tag="a_sb")
            for gi, x0 in enumerate(range(0, PL * HP, 512)):
                xn = min(512, PL * HP - x0)
                ups = psU.tile([W, 512], dt, tag="u", name="ups")
                nc.tensor.matmul(ups[:, :xn], mwT, ctxT[:, x0 : x0 + xn])
                alt_copy(gi, a_sb[:, x0 : x0 + xn], ups[:, :xn])

            if lvl < 6:
                _dbg_finish()
                continue

            # ---- upsample stage 2 + BN + relu, groups of 6 planes -------
            out_sb = big.tile([H, PL * W], dt, tag="out_sb")
            for gi in range(5):
                t2w = small.tile([HP, 6 * W], dt, tag="t2w", name="t2w")
                for j3 in range(2):
                    i0 = gi * 6 + 3 * j3
                    pool_, tag_ = (psS, "pss") if j3 == 0 else (psCK, "ck")
                    t2 = pool_.tile([HP, 3 * W], dt, tag=tag_, name="t2")
                    for k3 in range(3):
                        nc.tensor.transpose(
                            t2[:, k3 * W : (k3 + 1) * W],
                            a_sb[:, (i0 + k3) * HP : (i0 + k3 + 1) * HP],
                            id96,
                        )
                    alt_copy(j3, t2w[:, j3 * 3 * W : (j3 + 1) * 3 * W], t2[:])
                for half in range(2):
                    pool_, tag_ = (psU, "u") if (2 * gi + half) % 2 == 0 else (psA, "psA")
                    up = pool_.tile([H, 3 * W], dt, tag=tag_, name="up")
                    nc.tensor.matmul(
                        up[:], mhT, t2w[:, half * 3 * W : (half + 1) * 3 * W]
                    )
                    for j in range(3):
                        ig = gi * 6 + half * 3 + j
                        if ig % 2 == 0:
                            nc.scalar.activation(
                                out_sb[:, ig * W : (ig + 1) * W],
                                up[:, j * W : (j + 1) * W],
                                func=mybir.ActivationFunctionType.Relu,
                                bias=bnb[:, ig : ig + 1],
                                scale=1.0,
                            )
                        else:
                            nc.vector.scalar_tensor_tensor(
                                out_sb[:, ig * W : (ig + 1) * W],
                                up[:, j * W : (j + 1) * W],
                                bnb[:, ig : ig + 1], zeros_sb[:],
                                op0=mybir.AluOpType.add,
                                op1=mybir.AluOpType.max,
                            )
                (nc.sync if gi % 2 == 0 else nc.scalar).dma_start(
                    out3[:, gi * 576 : (gi + 1) * 576],
                    out_sb[:, gi * 576 : (gi + 1) * 576],
                )
        if barrier:
            nc.all_engine_barrier()

    _split_excess_waits(nc)
    return nc


_PROGRAM_CACHE = {}


def _get_program():
    if "nc" not in _PROGRAM_CACHE:
        _PROGRAM_CACHE["nc"] = _build_program()
    return _PROGRAM_CACHE["nc"]


def make_in_maps(p_fea, hu, coord_W, coord_b, query_W, query_b, key_W, key_b,
                 proj_W, bn_gamma, bn_beta, bn_mean, bn_var):
    p_fea = np.asarray(p_fea, np.float32)
    hu = np.asarray(hu, np.float32)
    coord_W = np.asarray(coord_W, np.float32)
    coord_b = np.asarray(coord_b, np.float32)
    query_W = np.asarray(query_W, np.float32)
    query_b = np.asarray(query_b, np.float32)
    key_W = np.asarray(key_W, np.float32)
    key_b = np.asarray(key_b, np.float32)
    proj_W = np.asarray(proj_W, np.float32)
    bn_gamma = np.asarray(bn_gamma, np.float32)
    bn_beta = np.asarray(bn_beta, np.float32)
    bn_mean = np.asarray(bn_mean, np.float32)
    bn_var = np.asarray(bn_var, np.float32)

    # ---- host constant folding ------------------------------------------
    cf9 = np.concatenate(
        [_coord_feats(HP, WP).reshape(8, X), np.ones((1, X), np.float32)],
        axis=0,
    )  # [9, 2304]; cf = A9 @ cf9
    A9 = np.concatenate([coord_W, coord_b[:, None]], axis=1)  # [10, 9]
    K9 = key_W[:, C:] @ A9
    K9[:, 8] += key_b
    Q9 = query_W[:, HID:] @ A9
    Q9[:, 8] += query_b
    Mh = _interp_matrix(H, HP)
    Mw = _interp_matrix(W, WP)
    bn_scale = bn_gamma / np.sqrt(bn_var + EPS)
    bn_bias = bn_beta - bn_mean * bn_scale
    WpS = bn_scale[:, :, None] * proj_W  # [parts, hid, c]
    qW_huT = query_W[:, :HID].T.copy()
    keyW_cT = key_W[:, :C].T.copy()

    stat = np.zeros((C, 70), np.float32)
    stat[:, 0:10] = keyW_cT
    for p in range(PARTS):
        stat[:, 10 + 10 * p : 20 + 10 * p] = WpS[p].T
    statcf = np.zeros((9, 70), np.float32)
    statcf[:, 0:10] = K9.T
    qhu_g = np.zeros((2, 64, 30), np.float32)
    for g in range(2):
        for j in range(PPC):
            qhu_g[g, 32 * g + 10 * j : 32 * g + 10 * j + 10,
                  10 * j : 10 * j + 10] = qW_huT
    qcf = np.zeros((9, 30), np.float32)
    for j in range(PPC):
        qcf[:, 10 * j : 10 * j + 10] = Q9.T
    mask3 = np.zeros((30, 30), np.float32)
    selM = np.zeros((30, 30), np.float32)
    for j in range(PPC):
        mask3[10 * j : 10 * j + 10, 10 * j : 10 * j + 10] = 1.0
        selM[10 * j : 10 * j + 10, 10 * j : 10 * j + 10] = np.eye(
            10, dtype=np.float32
        )

    in_maps = []
    for core in range(8):
        n_idx, s = core // 2, core % 2
        pset = [0, 1, 2] if s == 0 else [3, 4, 5]

        cbank = np.zeros((128, NCB), np.float32)
        cbank[:, 0:70] = stat[0:128]
        cbank[:, 70:140] = stat[128:256]
        cbank[0:96, 140:236] = np.eye(96, dtype=np.float32)
        cbank[0:WP, 236:332] = Mw.T
        cbank[0:HP, 332:428] = Mh.T
        cbank[0:70, 428:498] = np.eye(70, dtype=np.float32)
        cbank[0:64, 498:528] = qhu_g[0]
        cbank[0:64, 718:748] = qhu_g[1]
        for j, p in enumerate(pset):
            cbank[0:H, 528 + 10 * j : 528 + 10 * j + 10] = bn_bias[p][None, :]
        cbank[0:9, 558:628] = statcf
        cbank[0:9, 628:658] = qcf
        cbank[0:30, 658:688] = mask3
        cbank[0:30, 688:718] = selM

        ph = (p_fea[n_idx, :, 48 * s : 48 * s + 48, :]
              .reshape(2, 128, 48 * 96).transpose(1, 0, 2).reshape(128, 9216))
        hh = (hu[pset, n_idx].reshape(3, 10, 2, 48, 96)
              .transpose(2, 0, 1, 3, 4).reshape(2, 30, 48 * 96))
        hu4a = np.zeros((2, 32, 48 * 96), np.float32)
        hu4a[:, 0:30] = hh
        cf9ca = np.empty((9, X + XH), np.float32)
        cf9ca[:, 0:X] = cf9
        cf9ca[:, X : X + XH] = cf9[:, XH * s : XH * s + XH]

        in_maps.append({
            "pfe": np.ascontiguousarray(ph),
            "hu4": hu4a.reshape(64, 4608),
            "cbank": cbank,
            "cf9c": cf9ca,
        })
    return in_maps


def assemble_out(results):
    out = np.empty((PARTS, N, HID, H, W), np.float32)
    for core in range(8):
        n_idx, s = core // 2, core % 2
        pset = [0, 1, 2] if s == 0 else [3, 4, 5]
        r = results[core]["out3"].reshape(H, PPC, HID, W).transpose(1, 2, 0, 3)
        out[pset, n_idx] = r
    return out


def kernel(**inputs):
    in_maps = make_in_maps(**inputs)
    nc = _get_program()
    res = run_bass_kernel_spmd(nc, in_maps, core_ids=list(range(8)))
    return assemble_out(res.results)


# revision 31
# speedup vs baseline: 2.5000x; 2.5000x over previous
"""Trainium2 Bass kernel for nn_Dep_Context_80109730005366.

Math notes (exact restructurings of the reference):
  - ctx = (q @ key) @ value is reassociated as q @ (key @ value); KV is
    [hid, c] so the huge [hw, hw] energy matrix never materializes.
  - The 1x1 conv (proj_W) and the BN scale commute with the bilinear
    upsample, so we contract KV with proj_W into a per-part [hid, hid]
    matrix (KVW) and upsample 10 channels instead of 256.
  - Coord features are input-independent; everything derived from them
    (cf, key/query constant terms) is precomputed on host as tiny matrices.

Sharding: 8 cores = 4 batches x 2 half-part groups. Core k handles batch
n = k//2 and parts {0,1,2} (k even) or {3,4,5} (k odd). Shared per-batch
work (maxpool of p_fea, key, KVW) is duplicated across the 2 cores of a
batch; per-part work is split. (A pair ReduceScatter of partial KVW was
tried to halve the p_fea read, but tiny collectives serialize at ~23us
per op on this part — far more than the 2.4MB of HBM reads they save.)

Bandwidth: p_fea / hu / coord features are uploaded as bf16 (host-side
cast inside kernel()); the stat/q matmuls run bf16 with fp32 PSUM
accumulation and everything downstream of the PSUMs stays fp32. Max rel
error vs the fp32 reference is ~4e-3 (tolerance 2e-2).

Queue discipline: nc.sync carries ONLY input DMAs so that rep k+1's input
stream never queues behind rep k's late output DMAs; outputs go on
nc.scalar. Constants are DMA'd once, outside the rep loop.

Intermediates (keywp, kwT, q, ctxT, a, t2w) and the output are bf16 too;
only the PSUM accumulations and the tiny KVW selection stay fp32.

cbank (fp32): 0:30 bnb [96], 30:60 mask3_60 [60], 60:90 selM60 [60]
cbank_bf (bf16): 0:70 stat0 [128], 70:140 stat1 [128], 140:170 qhu_g0 [64],
  170:200 qhu_g1 [64], 200:270 statcf [9], 270:300 qcf [9],
  300:396 id96 [96] (top-left 70x70 doubles as id70),
  396:492 mwT [48], 492:588 mhT [48]
PSUM pools are split by phase: psA/psS/psK serve the front half (stat,
transposes, KVW, q), psB the back half (ctx, upsample) — so rep k+1's
front never waits on rep k's back-half PSUM slots.
"""

import ml_dtypes
import numpy as np

import bass_rust
import concourse.bass as bass
import concourse.tile as tile
from concourse import mybir
from concourse.bass_utils import run_bass_kernel_spmd
from concourse.vector_clock import ScopedClock

EPS = 1e-5
N, C, H, W = 4, 256, 96, 96
HP, WP = 48, 48
HID, PARTS = 10, 6
X = HP * WP        # 2304
PPC = 3            # parts per core
PL = PPC * HID     # planes per core = 30
NCB = 90           # fp32 cbank columns
NCBF = 588         # bf16 cbank columns
F32 = mybir.dt.float32
BF16 = mybir.dt.bfloat16

# ---------------------------------------------------------------------------
# Workaround: this container's walrus codegen rejects instructions carrying
# more than a couple of semaphore waits ("Too many sync wait commands").
# TileContext's exit path puts every outstanding wait on one Drain; spread
# them over a chain of single-wait nops instead.
# ---------------------------------------------------------------------------
_MAX_WAITS = 1


def _patched_drain_and_barrier(self, tick_clock, wait_clock):
    nc = self.nc
    drain_inst = nc.sync.drain()
    wait_clock.add_sem_waits(
        drain_inst.ins, ScopedClock({None: tick_clock.global_clock})
    )
    si = drain_inst.ins.sync_info
    waits = list(si.on_wait) if si is not None else []
    updates = list(si.on_update) if si is not None else []
    if len(waits) > _MAX_WAITS:
        drain_inst.ins.sync_info = bass_rust.SyncInfo(
            on_wait=waits[:_MAX_WAITS], on_update=updates
        )
        rest = waits[_MAX_WAITS:]
        for i in range(0, len(rest), _MAX_WAITS):
            nop = nc.sync.nop(nofuse=True, hint="split_drain_wait")
            nop.ins.sync_info = bass_rust.SyncInfo(
                on_wait=rest[i : i + _MAX_WAITS], on_update=[]
            )
    nc.all_engine_barrier()
    assert self.sems is not None
    popped = nc._tile_sem_poison_stack.pop()
    assert popped is self._sem_poison
    nc.clear_and_free_semaphores(list(self.sems.allocated().values()))
    nc.all_engine_barrier()


tile.TileContext._drain_and_barrier = _patched_drain_and_barrier

_BODY_MAX_WAITS = 1


def _split_excess_waits(nc, maxw=_BODY_MAX_WAITS):
    """Post-pass: any instruction carrying more than `maxw` semaphore waits
    gets the excess hoisted onto same-engine nops inserted right before it
    (the engine sequencer blocks on those first, preserving semantics)."""
    eng_map = {
        mybir.EngineType.SP: nc.sync,
        mybir.EngineType.PE: nc.tensor,
        mybir.EngineType.DVE: nc.vector,
        mybir.EngineType.Activation: nc.scalar,
        mybir.EngineType.Pool: nc.gpsimd,
    }

    def make_nop(engine_type, waits):
        bi = eng_map[engine_type].nop(nofuse=True, hint="wait_split")
        # pop it off the tail of the current bb; we'll splice it manually
        cur = nc.cur_bb.bb
        lst = cur.instructions
        assert lst[-1].name == bi.ins.name
        cur.instructions = lst[:-1]
        bi.ins.sync_info = bass_rust.SyncInfo(on_wait=waits, on_update=[])
        return bi.ins

    for bb in nc.m.functions[0].blocks:
        insts = bb.instructions
        out = []
        changed = False
        for inst in insts:
            si = inst.sync_info
            waits = list(si.on_wait) if si is not None else []
            if len(waits) > maxw:
                updates = list(si.on_update) if si is not None else []
                extra, keep = waits[:-maxw], waits[-maxw:]
                for j in range(0, len(extra), maxw):
                    out.append(make_nop(inst.engine, extra[j : j + maxw]))
                inst.sync_info = bass_rust.SyncInfo(on_wait=keep, on_update=updates)
                changed = True
            out.append(inst)
        if changed:
            bb.instructions = out


# ---------------------------------------------------------------------------
# Host-side constant precomputation (all tiny; heavy tensors stay on device)
# ---------------------------------------------------------------------------
def _coord_feats(hp, wp):
    xs = np.arange(wp, dtype=np.float32)
    ys = np.arange(hp, dtype=np.float32)
    xmin = xs / wp * 2 - 1
    xmax = (xs + 1) / wp * 2 - 1
    xctr = (xmin + xmax) / 2
    ymin = ys / hp * 2 - 1
    ymax = (ys + 1) / hp * 2 - 1
    yctr = (ymin + ymax) / 2
    Xb = lambda v: np.broadcast_to(v[None, :], (hp, wp))
    Yb = lambda v: np.broadcast_to(v[:, None], (hp, wp))
    ones = np.ones((hp, wp), np.float32)
    return np.stack(
        [Xb(xmin), Yb(ymin), Xb(xmax), Yb(ymax), Xb(xctr), Yb(yctr),
         ones / wp, ones / hp], axis=0,
    ).astype(np.float32)


def _interp_matrix(out_n, in_n):
    pos = np.arange(out_n, dtype=np.float32) * ((in_n - 1) / (out_n - 1))
    i0 = np.clip(np.floor(pos).astype(np.int64), 0, in_n - 1)
    i1 = np.clip(i0 + 1, 0, in_n - 1)
    w1 = (pos - i0).astype(np.float32)
    M = np.zeros((out_n, in_n), np.float32)
    for r in range(out_n):
        M[r, i0[r]] += 1 - w1[r]
        M[r, i1[r]] += w1[r]
    return M


# ---------------------------------------------------------------------------
# Device program (built once, shared SPMD across all 8 cores)
# ---------------------------------------------------------------------------
def _build_program(reps=1, barrier=False, use_collective=False, upto="full"):
    nc = bass.Bass()
    dt = F32
    STAGES = ["pool", "stat", "kvw", "q", "ctx", "ups1", "full"]
    lvl = STAGES.index(upto)

    pfe = nc.dram_tensor("pfe", [128, 2 * X * 4], BF16, kind="ExternalInput")
    hu4 = nc.dram_tensor("hu4", [64, 4608], BF16, kind="ExternalInput")
    cbank = nc.dram_tensor("cbank", [128, NCB], dt, kind="ExternalInput")
    cbank_bf = nc.dram_tensor("cbank_bf", [128, NCBF], BF16, kind="ExternalInput")
    cf9c = nc.dram_tensor("cf9c", [9, X], BF16, kind="ExternalInput")
    out3 = nc.dram_tensor("out3", [H, PL * W], BF16, kind="ExternalOutput")

    def alt_copy(idx, out, in_):
        # alternate psum->sbuf copies between ACT and DVE to balance engines
        if idx % 2 == 0:
            nc.scalar.copy(out, in_)
        else:
            nc.vector.tensor_copy(out, in_)

    with tile.TileContext(nc) as tc:
      with tc.tile_pool(name="glob", bufs=1) as glob:
        # ---- constants: loaded once, shared by all reps -----------------
        cb = glob.tile([128, NCB], dt, tag="cbank", name="cbank")
        nc.sync.dma_start(cb[:], cbank[:])
        cbf = glob.tile([128, NCBF], BF16, tag="cbank_bf", name="cbank_bf")
        nc.sync.dma_start(cbf[:], cbank_bf[:])
        cf9_sb = glob.tile([9, X], BF16, tag="cf9", name="cf9")
        nc.sync.dma_start(cf9_sb[:], cf9c[:])
        bnb = cb[0:H, 0:30]
        mask3_60 = cb[0:60, 30:60]
        selM60 = cb[0:60, 60:90]
        stat0 = cbf[0:128, 0:70]
        stat1 = cbf[0:128, 70:140]
        qhu_g = [cbf[0:64, 140:170], cbf[0:64, 170:200]]
        statcf = cbf[0:9, 200:270]
        qcf = cbf[0:9, 270:300]
        id96 = cbf[0:96, 300:396]
        id70 = cbf[0:70, 300:370]
        mwT = cbf[0:WP, 396:492]
        mhT = cbf[0:HP, 492:588]
        zeros_sb = glob.tile([H, W], dt, tag="zeros", name="zeros")
        nc.gpsimd.memset(zeros_sb[:], 0.0)

        for _rep in range(reps):
          with (
            tc.tile_pool(name="pfe_in", bufs=4) as pfe_pool,
            tc.tile_pool(name="p1", bufs=2) as p1_pool,
            tc.tile_pool(name="pf", bufs=2) as pf_pool,
            tc.tile_pool(name="hu", bufs=2) as hu_pool,
            tc.tile_pool(name="big", bufs=2) as big,
            tc.tile_pool(name="small", bufs=2) as small,
            tc.tile_pool(name="psA", bufs=2, space="PSUM") as psA,
            tc.tile_pool(name="psS", bufs=2, space="PSUM") as psS,
            tc.tile_pool(name="psK", bufs=1, space="PSUM") as psK,
            tc.tile_pool(name="psB", bufs=3, space="PSUM") as psB,
          ):
            # ---- input DMAs (all on nc.sync: inputs-only queue) ---------
            pf_t = [
                pf_pool.tile([128, HP, WP], BF16, tag=f"pf{t}", name=f"pf{t}")
                for t in range(2)
            ]
            chunks = []
            for c_i in range(16):
                hh8, t = c_i // 2, c_i % 2
                ch = pfe_pool.tile([128, 12, 96], BF16, tag="chunk", name="chunk")
                nc.sync.dma_start(
                    ch[:],
                    pfe[:, t * 9216 + hh8 * 1152 : t * 9216 + (hh8 + 1) * 1152]
                    .rearrange("c (r w) -> c r w", w=96),
                )
                chunks.append((hh8, t, ch))
                if c_i == 3:
                    hu_sb = hu_pool.tile([64, 48, 96], BF16, tag="hu_sb")
                    nc.sync.dma_start(
                        hu_sb[:], hu4.rearrange("p (r w) -> p r w", w=96)
                    )

            # ---- p_fea maxpool (h-pairs first: dense stage1) ------------
            for hh8, t, ch in chunks:
                p1 = p1_pool.tile([128, 6, 96], BF16, tag="p1", name="p1")
                ch2 = ch.rearrange("c (h2 two) w -> c h2 two w", two=2)
                nc.vector.tensor_max(p1[:], ch2[:, :, 0, :], ch2[:, :, 1, :])
                p1w = p1.rearrange("c h (w2 two) -> c h w2 two", two=2)
                nc.vector.tensor_max(
                    pf_t[t][:, hh8 * 6 : (hh8 + 1) * 6, :],
                    p1w[:, :, :, 0],
                    p1w[:, :, :, 1],
                )

            # ---- hu maxpool (h-pairs first) -----------------------------
            hu1 = hu_pool.tile([64, 24, 96], BF16, tag="hu1")
            hv = hu_sb.rearrange("p (h2 two) w -> p h2 two w", two=2)
            nc.vector.tensor_max(hu1[:], hv[:, :, 0, :], hv[:, :, 1, :])
            hup = hu_pool.tile([64, 24, 48], BF16, tag="hup")
            h1w = hu1.rearrange("p h (w2 two) -> p h w2 two", two=2)
            nc.vector.tensor_max(hup[:], h1w[:, :, :, 0], h1w[:, :, :, 1])
            hup_f = hup.rearrange("p h w -> p (h w)")

            # ---- debug early-exit: zero-fill output and stop ------------
            out_sb0 = None
            if lvl < 6:
                out_sb0 = big.tile([H, PL * W], BF16, tag="out_sb")
                nc.gpsimd.memset(out_sb0[:], 0.0)

            def _dbg_finish():
                nc.scalar.dma_start(out3[:], out_sb0[:])

            if lvl < 1:
                _dbg_finish()
                continue

            # ---- key|WpPf stat matmul over pooled p_fea -----------------
            keywp = big.tile([70, X], BF16, tag="keywp")
            pf_f = [t_.rearrange("c h w -> c (h w)") for t_ in pf_t]
            for xi in range(8):
                x0 = xi * 288
                ps = psA.tile([70, 288], dt, tag="psA", name="ps")
                nc.tensor.matmul(
                    ps[:], stat0, pf_f[0][:, x0 : x0 + 288],
                    start=True, stop=False,
                )
                nc.tensor.matmul(
                    ps[:], stat1, pf_f[1][:, x0 : x0 + 288],
                    start=False, stop=False,
                )
                nc.tensor.matmul(
                    ps[:], statcf, cf9_sb[:, x0 : x0 + 288],
                    start=False, stop=True,
                )
                alt_copy(xi, keywp[:, x0 : x0 + 288], ps[:])

            if lvl < 2:
                _dbg_finish()
                continue

            # ---- transpose -> KVW^T [60, 10] (all 6 parts) --------------
            kwT = big.tile([128, 18, 70], BF16, tag="kwT")
            for b in range(18):
                tp = psS.tile([128, 70], BF16, tag="pss", name="tp")
                nc.tensor.transpose(
                    tp[:], keywp[:, b * 128 : (b + 1) * 128], id70
                )
                alt_copy(b, kwT[:, b, :], tp[:])
            kvwT_ps = psK.tile([60, 10], dt, tag="ck", name="kvwT_ps")
            for b in range(18):
                nc.tensor.matmul(
                    kvwT_ps[:], kwT[:, b, 10:70], kwT[:, b, 0:10],
                    start=(b == 0), stop=(b == 17),
                )
            kvwT_sb = small.tile([60, 10], dt, tag="kvwT_sb")
            nc.vector.tensor_copy(kvwT_sb[:], kvwT_ps[:])

            if lvl < 3:
                _dbg_finish()
                continue

            # ---- q matmuls ----------------------------------------------
            q_sb = big.tile([PL, X], BF16, tag="q_sb")
            for c_ in range(8):
                g, hx = c_ // 4, (c_ % 4) * 288
                x0 = g * 1152 + hx
                ps = psA.tile([PL, 288], dt, tag="psA", name="ps")
                nc.tensor.matmul(
                    ps[:], qhu_g[g], hup_f[:, hx : hx + 288],
                    start=True, stop=False,
                )
                nc.tensor.matmul(
                    ps[:], qcf, cf9_sb[:, x0 : x0 + 288], start=False, stop=True
                )
                alt_copy(c_ + 1, q_sb[:, x0 : x0 + 288], ps[:])

            if lvl < 4:
                _dbg_finish()
                continue

            # ---- block-diag KVW (this core's 3 parts) -------------------
            S_sb = small.tile([60, PL], dt, tag="S_sb")
            for j in range(3):
                nc.vector.tensor_mul(
                    S_sb[:, j * 10 : (j + 1) * 10], kvwT_sb[:],
                    mask3_60[:, j * 10 : (j + 1) * 10],
                )
            kvwbd_ps = psK.tile([PL, PL], dt, tag="ck", name="kvwbd_ps")
            nc.tensor.matmul(kvwbd_ps[:], S_sb[:], selM60)
            kvwbd = small.tile([PL, PL], BF16, tag="kvwbd")
            nc.scalar.copy(kvwbd[:], kvwbd_ps[:])

            # ---- ctx, transposed, plane-major free layout ---------------
            q3 = q_sb.rearrange("p (h w) -> p h w", h=HP)
            ctxT = big.tile([WP, PL * HP], BF16, tag="ctxT")
            ctxT_hi = ctxT.rearrange("w (i h) -> w h i", i=PL)
            for g3 in range(3):
                cps = psB.tile([WP, 16 * PL], dt, tag="psb", name="cps")
                for hh in range(16):
                    hp_i = g3 * 16 + hh
                    nc.tensor.matmul(
                        cps[:, hh * PL : (hh + 1) * PL],
                        q3[:, hp_i, :],
                        kvwbd[:],
                    )
                cps_v = cps.rearrange("w (h i) -> w h i", h=16)
                alt_copy(g3, ctxT_hi[:, g3 * 16 : (g3 + 1) * 16, :], cps_v[:])

            if lvl < 5:
                _dbg_finish()
                continue

            # ---- upsample stage 1: contract w' --------------------------
            a_sb = big.tile([W, PL * HP], BF16, tag="a_sb")
            for gi, x0 in enumerate(range(0, PL * HP, 512)):
                xn = min(512, PL * HP - x0)
                ups = psB.tile([W, 512], dt, tag="psb", name="ups")
                nc.tensor.matmul(ups[:, :xn], mwT, ctxT[:, x0 : x0 + xn])
                alt_copy(gi, a_sb[:, x0 : x0 + xn], ups[:, :xn])

            if lvl < 6:
                _dbg_finish()
                continue

            # ---- upsample stage 2 + BN + relu, groups of 6 planes -------
            out_sb = big.tile([H, PL * W], BF16, tag="out_sb")
            for gi in range(5):
                t2w = small.tile([HP, 6 * W], BF16, tag="t2w", name="t2w")
                for j3 in range(2):
                    i0 = gi * 6 + 3 * j3
                    t2 = psB.tile([HP, 3 * W], BF16, tag="psb", name="t2")
                    for k3 in range(3):
                        nc.tensor.transpose(
                            t2[:, k3 * W : (k3 + 1) * W],
                            a_sb[:, (i0 + k3) * HP : (i0 + k3 + 1) * HP],
                            id96,
                        )
                    alt_copy(j3, t2w[:, j3 * 3 * W : (j3 + 1) * 3 * W], t2[:])
                for half in range(2):
                    up = psB.tile([H, 3 * W], dt, tag="psb", name="up")
                    nc.tensor.matmul(
                        up[:], mhT, t2w[:, half * 3 * W : (half + 1) * 3 * W]
                    )
                    for j in range(3):
                        ig = gi * 6 + half * 3 + j
                        if ig % 3 != 2:
                            nc.scalar.activation(
                                out_sb[:, ig * W : (ig + 1) * W],
                                up[:, j * W : (j + 1) * W],
                                func=mybir.ActivationFunctionType.Relu,
                                bias=bnb[:, ig : ig + 1],
                                scale=1.0,
                            )
                        else:
                            nc.vector.scalar_tensor_tensor(
                                out_sb[:, ig * W : (ig + 1) * W],
                                up[:, j * W : (j + 1) * W],
                                bnb[:, ig : ig + 1], zeros_sb[:],
                                op0=mybir.AluOpType.add,
                                op1=mybir.AluOpType.max,
                            )
                nc.scalar.dma_start(
                    out3[:, gi * 576 : (gi + 1) * 576],
                    out_sb[:, gi * 576 : (gi + 1) * 576],
                )
          if barrier:
            nc.all_engine_barrier()

    _split_excess_waits(nc)
    return nc


_PROGRAM_CACHE = {}


def _get_program():
    if "nc" not in _PROGRAM_CACHE:
        _PROGRAM_CACHE["nc"] = _build_program()
    return _PROGRAM_CACHE["nc"]


def make_in_maps(p_fea, hu, coord_W, coord_b, query_W, query_b, key_W, key_b,
                 proj_W, bn_gamma, bn_beta, bn_mean, bn_var):
    p_fea = np.asarray(p_fea, np.float32)
    hu = np.asarray(hu, np.float32)
    coord_W = np.asarray(coord_W, np.float32)
    coord_b = np.asarray(coord_b, np.float32)
    query_W = np.asarray(query_W, np.float32)
    query_b = np.asarray(query_b, np.float32)
    key_W = np.asarray(key_W, np.float32)
    key_b = np.asarray(key_b, np.float32)
    proj_W = np.asarray(proj_W, np.float32)
    bn_gamma = np.asarray(bn_gamma, np.float32)
    bn_beta = np.asarray(bn_beta, np.float32)
    bn_mean = np.asarray(bn_mean, np.float32)
    bn_var = np.asarray(bn_var, np.float32)

    # ---- host constant folding ------------------------------------------
    cf9 = np.concatenate(
        [_coord_feats(HP, WP).reshape(8, X), np.ones((1, X), np.float32)],
        axis=0,
    )  # [9, 2304]; cf = A9 @ cf9
    A9 = np.concatenate([coord_W, coord_b[:, None]], axis=1)  # [10, 9]
    K9 = key_W[:, C:] @ A9
    K9[:, 8] += key_b
    Q9 = query_W[:, HID:] @ A9
    Q9[:, 8] += query_b
    Mh = _interp_matrix(H, HP)
    Mw = _interp_matrix(W, WP)
    bn_scale = bn_gamma / np.sqrt(bn_var + EPS)
    bn_bias = bn_beta - bn_mean * bn_scale
    WpS = bn_scale[:, :, None] * proj_W  # [parts, hid, c]
    qW_huT = query_W[:, :HID].T.copy()
    keyW_cT = key_W[:, :C].T.copy()

    stat = np.zeros((C, 70), np.float32)
    stat[:, 0:10] = keyW_cT
    for p in range(PARTS):
        stat[:, 10 + 10 * p : 20 + 10 * p] = WpS[p].T
    statcf = np.zeros((9, 70), np.float32)
    statcf[:, 0:10] = K9.T
    qhu_g = np.zeros((2, 64, 30), np.float32)
    for g in range(2):
        for j in range(PPC):
            qhu_g[g, 32 * g + 10 * j : 32 * g + 10 * j + 10,
                  10 * j : 10 * j + 10] = qW_huT
    qcf = np.zeros((9, 30), np.float32)
    for j in range(PPC):
        qcf[:, 10 * j : 10 * j + 10] = Q9.T

    cbank_bf = np.zeros((128, NCBF), np.float32)
    cbank_bf[:, 0:70] = stat[0:128]
    cbank_bf[:, 70:140] = stat[128:256]
    cbank_bf[0:64, 140:170] = qhu_g[0]
    cbank_bf[0:64, 170:200] = qhu_g[1]
    cbank_bf[0:9, 200:270] = statcf
    cbank_bf[0:9, 270:300] = qcf
    cbank_bf[0:96, 300:396] = np.eye(96, dtype=np.float32)
    cbank_bf[0:WP, 396:492] = Mw.T
    cbank_bf[0:HP, 492:588] = Mh.T
    cbank_bf = cbank_bf.astype(ml_dtypes.bfloat16)

    cf9_bf = np.ascontiguousarray(cf9).astype(ml_dtypes.bfloat16)

    in_maps = []
    for core in range(8):
        n_idx, s = core // 2, core % 2
        pset = [0, 1, 2] if s == 0 else [3, 4, 5]

        cbank = np.zeros((128, NCB), np.float32)
        for j, p in enumerate(pset):
            cbank[0:H, 10 * j : 10 * j + 10] = bn_bias[p][None, :]
        # mask3_60 / selM60: select this core's parts out of the 6-part KVW^T
        for j in range(PPC):
            p6 = 3 * s + j
            cbank[10 * p6 : 10 * p6 + 10, 30 + 10 * j : 40 + 10 * j] = 1.0
            cbank[10 * p6 : 10 * p6 + 10, 60 + 10 * j : 70 + 10 * j] = np.eye(
                10, dtype=np.float32
            )

        ph = (p_fea[n_idx].reshape(2, 128, X * 4)
              .transpose(1, 0, 2).reshape(128, 2 * X * 4))
        hh = (hu[pset, n_idx].reshape(3, 10, 2, 48, 96)
              .transpose(2, 0, 1, 3, 4).reshape(2, 30, 48 * 96))
        hu4a = np.zeros((2, 32, 48 * 96), np.float32)
        hu4a[:, 0:30] = hh

        in_maps.append({
            "pfe": np.ascontiguousarray(ph).astype(ml_dtypes.bfloat16),
            "hu4": hu4a.reshape(64, 4608).astype(ml_dtypes.bfloat16),
            "cbank": cbank,
            "cbank_bf": cbank_bf,
            "cf9c": cf9_bf,
        })
    return in_maps


def assemble_out(results):
    out = np.empty((PARTS, N, HID, H, W), np.float32)
    for core in range(8):
        n_idx, s = core // 2, core % 2
        pset = [0, 1, 2] if s == 0 else [3, 4, 5]
        r = (results[core]["out3"].astype(np.float32)
             .reshape(H, PPC, HID, W).transpose(1, 2, 0, 3))
        out[pset, n_idx] = r
    return out


def kernel(**inputs):
    in_maps = make_in_maps(**inputs)
    nc = _get_program()
    res = run_bass_kernel_spmd(nc, in_maps, core_ids=list(range(8)))
    return assemble_out(res.results)


# revision 36
# speedup vs baseline: 3.7716x; 1.5086x over previous
"""Trainium2 Bass kernel for nn_Dep_Context_80109730005366.

Math notes (exact restructurings of the reference):
  - ctx = (q @ key) @ value is reassociated as q @ (key @ value); KV is
    [hid, c] so the huge [hw, hw] energy matrix never materializes.
  - The 1x1 conv (proj_W) and the BN scale commute with the bilinear
    upsample, so we contract KV with proj_W into a per-part [hid, hid]
    matrix (KVW) and upsample 10 channels instead of 256.
  - Coord features are input-independent; everything derived from them
    (cf, key/query constant terms) is precomputed on host as tiny matrices.

Sharding: 8 cores = 4 batches x 2 half-part groups. Core k handles batch
n = k//2 and parts {0,1,2} (k even) or {3,4,5} (k odd). Shared per-batch
work (maxpool of p_fea, key, KVW) is duplicated across the 2 cores of a
batch; per-part work is split. (A pair ReduceScatter of partial KVW was
tried to halve the p_fea read, but tiny collectives serialize at ~23us
per op on this part — far more than the 2.4MB of HBM reads they save.)

Bandwidth: p_fea / hu / coord features are uploaded as bf16 (host-side
cast inside kernel()); the stat/q matmuls run bf16 with fp32 PSUM
accumulation and everything downstream of the PSUMs stays fp32. Max rel
error vs the fp32 reference is ~4e-3 (tolerance 2e-2).

Queue discipline: nc.sync carries ONLY input DMAs so that rep k+1's input
stream never queues behind rep k's late output DMAs; outputs go on
nc.scalar. Constants are DMA'd once, outside the rep loop.

Intermediates (keywp, kwT, q, ctxT, a, t2w) and the output are bf16 too;
only the PSUM accumulations and the tiny KVW selection stay fp32.

cbank (fp32): 0:30 bnb [96], 30:60 mask3_60 [60], 60:90 selM60 [60]
cbank_bf (bf16): 0:70 stat0 [128], 70:140 stat1 [128],
  140:260 qhu_g0..3 [128] (query stationaries zero outside row-group g --
  PE operands must sit at base partition 0, so hu uses all 128 partitions
  as 4 row-groups and each group's stationary masks the others),
  260:330 statcf [9], 330:360 qcf [9],
  360:456 id96 [96] (top-left 70x70 doubles as id70),
  456:552 mwT [48], 552:648 mhT [48]
PSUM pools are split by phase: psA/psS/psK serve the front half (stat,
transposes, KVW, q), psB the back half (ctx, upsample) — so rep k+1's
front never waits on rep k's back-half PSUM slots.
"""

import ml_dtypes
import numpy as np

import bass_rust
import concourse.bass as bass
import concourse.tile as tile
from concourse import mybir
from concourse.bass_utils import run_bass_kernel_spmd
from concourse.vector_clock import ScopedClock

EPS = 1e-5
N, C, H, W = 4, 256, 96, 96
HP, WP = 48, 48
HID, PARTS = 10, 6
X = HP * WP        # 2304
PPC = 3            # parts per core
PL = PPC * HID     # planes per core = 30
NCB = 90           # fp32 cbank columns
NCBF = 648         # bf16 cbank columns
F32 = mybir.dt.float32
BF16 = mybir.dt.bfloat16

# ---------------------------------------------------------------------------
# Workaround: this container's walrus codegen rejects instructions carrying
# more than a couple of semaphore waits ("Too many sync wait commands").
# TileContext's exit path puts every outstanding wait on one Drain; spread
# them over a chain of single-wait nops instead.
# ---------------------------------------------------------------------------
_MAX_WAITS = 1


def _patched_drain_and_barrier(self, tick_clock, wait_clock):
    nc = self.nc
    drain_inst = nc.sync.drain()
    wait_clock.add_sem_waits(
        drain_inst.ins, ScopedClock({None: tick_clock.global_clock})
    )
    si = drain_inst.ins.sync_info
    waits = list(si.on_wait) if si is not None else []
    updates = list(si.on_update) if si is not None else []
    if len(waits) > _MAX_WAITS:
        drain_inst.ins.sync_info = bass_rust.SyncInfo(
            on_wait=waits[:_MAX_WAITS], on_update=updates
        )
        rest = waits[_MAX_WAITS:]
        for i in range(0, len(rest), _MAX_WAITS):
            nop = nc.sync.nop(nofuse=True, hint="split_drain_wait")
            nop.ins.sync_info = bass_rust.SyncInfo(
                on_wait=rest[i : i + _MAX_WAITS], on_update=[]
            )
    nc.all_engine_barrier()
    assert self.sems is not None
    popped = nc._tile_sem_poison_stack.pop()
    assert popped is self._sem_poison
    nc.clear_and_free_semaphores(list(self.sems.allocated().values()))
    nc.all_engine_barrier()


tile.TileContext._drain_and_barrier = _patched_drain_and_barrier

_BODY_MAX_WAITS = 1


def _split_excess_waits(nc, maxw=_BODY_MAX_WAITS):
    """Post-pass: any instruction carrying more than `maxw` semaphore waits
    gets the excess hoisted onto same-engine nops inserted right before it
    (the engine sequencer blocks on those first, preserving semantics)."""
    eng_map = {
        mybir.EngineType.SP: nc.sync,
        mybir.EngineType.PE: nc.tensor,
        mybir.EngineType.DVE: nc.vector,
        mybir.EngineType.Activation: nc.scalar,
        mybir.EngineType.Pool: nc.gpsimd,
    }

    def make_nop(engine_type, waits):
        bi = eng_map[engine_type].nop(nofuse=True, hint="wait_split")
        # pop it off the tail of the current bb; we'll splice it manually
        cur = nc.cur_bb.bb
        lst = cur.instructions
        assert lst[-1].name == bi.ins.name
        cur.instructions = lst[:-1]
        bi.ins.sync_info = bass_rust.SyncInfo(on_wait=waits, on_update=[])
        return bi.ins

    for bb in nc.m.functions[0].blocks:
        insts = bb.instructions
        out = []
        changed = False
        for inst in insts:
            si = inst.sync_info
            waits = list(si.on_wait) if si is not None else []
            if len(waits) > maxw:
                updates = list(si.on_update) if si is not None else []
                extra, keep = waits[:-maxw], waits[-maxw:]
                for j in range(0, len(extra), maxw):
                    out.append(make_nop(inst.engine, extra[j : j + maxw]))
                inst.sync_info = bass_rust.SyncInfo(on_wait=keep, on_update=updates)
                changed = True
            out.append(inst)
        if changed:
            bb.instructions = out


# ---------------------------------------------------------------------------
# Host-side constant precomputation (all tiny; heavy tensors stay on device)
# ---------------------------------------------------------------------------
def _coord_feats(hp, wp):
    xs = np.arange(wp, dtype=np.float32)
    ys = np.arange(hp, dtype=np.float32)
    xmin = xs / wp * 2 - 1
    xmax = (xs + 1) / wp * 2 - 1
    xctr = (xmin + xmax) / 2
    ymin = ys / hp * 2 - 1
    ymax = (ys + 1) / hp * 2 - 1
    yctr = (ymin + ymax) / 2
    Xb = lambda v: np.broadcast_to(v[None, :], (hp, wp))
    Yb = lambda v: np.broadcast_to(v[:, None], (hp, wp))
    ones = np.ones((hp, wp), np.float32)
    return np.stack(
        [Xb(xmin), Yb(ymin), Xb(xmax), Yb(ymax), Xb(xctr), Yb(yctr),
         ones / wp, ones / hp], axis=0,
    ).astype(np.float32)


def _interp_matrix(out_n, in_n):
    pos = np.arange(out_n, dtype=np.float32) * ((in_n - 1) / (out_n - 1))
    i0 = np.clip(np.floor(pos).astype(np.int64), 0, in_n - 1)
    i1 = np.clip(i0 + 1, 0, in_n - 1)
    w1 = (pos - i0).astype(np.float32)
    M = np.zeros((out_n, in_n), np.float32)
    for r in range(out_n):
        M[r, i0[r]] += 1 - w1[r]
        M[r, i1[r]] += w1[r]
    return M


# ---------------------------------------------------------------------------
# Device program (built once, shared SPMD across all 8 cores)
# ---------------------------------------------------------------------------
def _build_program(reps=1, barrier=False, use_collective=False, upto="full"):
    nc = bass.Bass()
    dt = F32
    STAGES = ["pool", "stat", "kvw", "q", "ctx", "ups1", "full"]
    lvl = STAGES.index(upto)

    pfe = nc.dram_tensor("pfe", [128, 2 * X * 4], BF16, kind="ExternalInput")
    hu4 = nc.dram_tensor("hu4", [128, 2304], BF16, kind="ExternalInput")
    cbank = nc.dram_tensor("cbank", [128, NCB], dt, kind="ExternalInput")
    cbank_bf = nc.dram_tensor("cbank_bf", [128, NCBF], BF16, kind="ExternalInput")
    cf9c = nc.dram_tensor("cf9c", [9, X], BF16, kind="ExternalInput")
    out3 = nc.dram_tensor("out3", [H, PL * W], BF16, kind="ExternalOutput")

    def alt_copy(idx, out, in_):
        # alternate psum->sbuf copies between ACT and DVE to balance engines
        if idx % 2 == 0:
            nc.scalar.copy(out, in_)
        else:
            nc.vector.tensor_copy(out, in_)

    with tile.TileContext(nc) as tc:
      with tc.tile_pool(name="glob", bufs=1) as glob:
        # ---- constants: loaded once, shared by all reps -----------------
        cb = glob.tile([128, NCB], dt, tag="cbank", name="cbank")
        nc.sync.dma_start(cb[:], cbank[:])
        cbf = glob.tile([128, NCBF], BF16, tag="cbank_bf", name="cbank_bf")
        nc.sync.dma_start(cbf[:], cbank_bf[:])
        cf9_sb = glob.tile([9, X], BF16, tag="cf9", name="cf9")
        nc.sync.dma_start(cf9_sb[:], cf9c[:])
        bnb = cb[0:H, 0:30]
        mask3_60 = cb[0:60, 30:60]
        selM60 = cb[0:60, 60:90]
        stat0 = cbf[0:128, 0:70]
        stat1 = cbf[0:128, 70:140]
        qhu_g = [cbf[0:128, 140 + 30 * g : 170 + 30 * g] for g in range(4)]
        statcf = cbf[0:9, 260:330]
        qcf = cbf[0:9, 330:360]
        id96 = cbf[0:96, 360:456]
        id70 = cbf[0:70, 360:430]
        mwT = cbf[0:WP, 456:552]
        mhT = cbf[0:HP, 552:648]
        zeros_sb = glob.tile([H, W], dt, tag="zeros", name="zeros")
        nc.gpsimd.memset(zeros_sb[:], 0.0)

        for _rep in range(reps):
          with (
            tc.tile_pool(name="pfe_in", bufs=6) as pfe_pool,
            tc.tile_pool(name="p1", bufs=3) as p1_pool,
            tc.tile_pool(name="pf", bufs=3) as pf_pool,
            tc.tile_pool(name="hu", bufs=3) as hu_pool,
            tc.tile_pool(name="big", bufs=3) as big,
            tc.tile_pool(name="small", bufs=3) as small,
            tc.tile_pool(name="psA", bufs=2, space="PSUM") as psA,
            tc.tile_pool(name="psS", bufs=2, space="PSUM") as psS,
            tc.tile_pool(name="psK", bufs=1, space="PSUM") as psK,
            tc.tile_pool(name="psB", bufs=3, space="PSUM") as psB,
          ):
            # ---- input DMAs (all on nc.sync: inputs-only queue) ---------
            pf_t = [
                pf_pool.tile([128, HP, WP], BF16, tag=f"pf{t}", name=f"pf{t}")
                for t in range(2)
            ]
            chunks = []
            for c_i in range(16):
                hh8, t = c_i // 2, c_i % 2
                ch = pfe_pool.tile([128, 12, 96], BF16, tag="chunk", name="chunk")
                nc.sync.dma_start(
                    ch[:],
                    pfe[:, t * 9216 + hh8 * 1152 : t * 9216 + (hh8 + 1) * 1152]
                    .rearrange("c (r w) -> c r w", w=96),
                )
                chunks.append((hh8, t, ch))
                if c_i == 3:
                    hu_sb = hu_pool.tile([128, 24, 96], BF16, tag="hu_sb")
                    nc.sync.dma_start(
                        hu_sb[:], hu4.rearrange("p (r w) -> p r w", w=96)
                    )

            # ---- p_fea maxpool (h-pairs first: dense stage1) ------------
            for hh8, t, ch in chunks:
                p1 = p1_pool.tile([128, 6, 96], BF16, tag="p1", name="p1")
                ch2 = ch.rearrange("c (h2 two) w -> c h2 two w", two=2)
                nc.vector.tensor_max(p1[:], ch2[:, :, 0, :], ch2[:, :, 1, :])
                p1w = p1.rearrange("c h (w2 two) -> c h w2 two", two=2)
                nc.vector.tensor_max(
                    pf_t[t][:, hh8 * 6 : (hh8 + 1) * 6, :],
                    p1w[:, :, :, 0],
                    p1w[:, :, :, 1],
                )

            # ---- hu maxpool (h-pairs first) -----------------------------
            hu1 = hu_pool.tile([128, 12, 96], BF16, tag="hu1")
            hv = hu_sb.rearrange("p (h2 two) w -> p h2 two w", two=2)
            nc.vector.tensor_max(hu1[:], hv[:, :, 0, :], hv[:, :, 1, :])
            hup = hu_pool.tile([128, 12, 48], BF16, tag="hup")
            h1w = hu1.rearrange("p h (w2 two) -> p h w2 two", two=2)
            nc.vector.tensor_max(hup[:], h1w[:, :, :, 0], h1w[:, :, :, 1])
            hup_f = hup.rearrange("p h w -> p (h w)")

            # ---- debug early-exit: zero-fill output and stop ------------
            out_sb0 = None
            if lvl < 6:
                out_sb0 = big.tile([H, PL * W], BF16, tag="out_sb")
                nc.gpsimd.memset(out_sb0[:], 0.0)

            def _dbg_finish():
                nc.scalar.dma_start(out3[:], out_sb0[:])

            if lvl < 1:
                _dbg_finish()
                continue

            # ---- key|WpPf stat matmul over pooled p_fea -----------------
            keywp = big.tile([70, X], BF16, tag="keywp")
            pf_f = [t_.rearrange("c h w -> c (h w)") for t_ in pf_t]
            for xi in range(8):
                x0 = xi * 288
                ps = psA.tile([70, 288], dt, tag="psA", name="ps")
                nc.tensor.matmul(
                    ps[:], stat0, pf_f[0][:, x0 : x0 + 288],
                    start=True, stop=False,
                )
                nc.tensor.matmul(
                    ps[:], stat1, pf_f[1][:, x0 : x0 + 288],
                    start=False, stop=False,
                )
                nc.tensor.matmul(
                    ps[:], statcf, cf9_sb[:, x0 : x0 + 288],
                    start=False, stop=True,
                )
                alt_copy(xi, keywp[:, x0 : x0 + 288], ps[:])

            if lvl < 2:
                _dbg_finish()
                continue

            # ---- transpose -> KVW^T [60, 10] (all 6 parts) --------------
            kwT = big.tile([128, 18, 70], BF16, tag="kwT")
            for b in range(18):
                tp = psS.tile([128, 70], BF16, tag="pss", name="tp")
                nc.tensor.transpose(
                    tp[:], keywp[:, b * 128 : (b + 1) * 128], id70
                )
                alt_copy(b, kwT[:, b, :], tp[:])
            kvwT_ps = psK.tile([60, 10], dt, tag="ck", name="kvwT_ps")
            for b in range(18):
                nc.tensor.matmul(
                    kvwT_ps[:], kwT[:, b, 10:70], kwT[:, b, 0:10],
                    start=(b == 0), stop=(b == 17),
                )
            kvwT_sb = small.tile([60, 10], dt, tag="kvwT_sb")
            nc.vector.tensor_copy(kvwT_sb[:], kvwT_ps[:])

            if lvl < 3:
                _dbg_finish()
                continue

            # ---- q matmuls ----------------------------------------------
            q_sb = big.tile([PL, X], BF16, tag="q_sb")
            for c_ in range(8):
                g, hx = c_ // 2, (c_ % 2) * 288
                x0 = g * 576 + hx
                ps = psA.tile([PL, 288], dt, tag="psA", name="ps")
                nc.tensor.matmul(
                    ps[:], qhu_g[g], hup_f[:, hx : hx + 288],
                    start=True, stop=False,
                )
                nc.tensor.matmul(
                    ps[:], qcf, cf9_sb[:, x0 : x0 + 288], start=False, stop=True
                )
                alt_copy(c_ + 1, q_sb[:, x0 : x0 + 288], ps[:])

            if lvl < 4:
                _dbg_finish()
                continue

            # ---- block-diag KVW (this core's 3 parts) -------------------
            S_sb = small.tile([60, PL], dt, tag="S_sb")
            for j in range(3):
                nc.vector.tensor_mul(
                    S_sb[:, j * 10 : (j + 1) * 10], kvwT_sb[:],
                    mask3_60[:, j * 10 : (j + 1) * 10],
                )
            kvwbd_ps = psK.tile([PL, PL], dt, tag="ck", name="kvwbd_ps")
            nc.tensor.matmul(kvwbd_ps[:], S_sb[:], selM60)
            kvwbd = small.tile([PL, PL], BF16, tag="kvwbd")
            nc.scalar.copy(kvwbd[:], kvwbd_ps[:])

            # ---- ctx, transposed, plane-major free layout ---------------
            q3 = q_sb.rearrange("p (h w) -> p h w", h=HP)
            ctxT = big.tile([WP, PL * HP], BF16, tag="ctxT")
            ctxT_hi = ctxT.rearrange("w (i h) -> w h i", i=PL)
            for g3 in range(3):
                cps = psB.tile([WP, 16 * PL], dt, tag="psb", name="cps")
                for hh in range(16):
                    hp_i = g3 * 16 + hh
                    nc.tensor.matmul(
                        cps[:, hh * PL : (hh + 1) * PL],
                        q3[:, hp_i, :],
                        kvwbd[:],
                    )
                cps_v = cps.rearrange("w (h i) -> w h i", h=16)
                alt_copy(g3, ctxT_hi[:, g3 * 16 : (g3 + 1) * 16, :], cps_v[:])

            if lvl < 5:
                _dbg_finish()
                continue

            # ---- upsample stage 1: contract w' --------------------------
            a_sb = big.tile([W, PL * HP], BF16, tag="a_sb")
            for gi, x0 in enumerate(range(0, PL * HP, 512)):
                xn = min(512, PL * HP - x0)
                ups = psB.tile([W, 512], dt, tag="psb", name="ups")
                nc.tensor.matmul(ups[:, :xn], mwT, ctxT[:, x0 : x0 + xn])
                alt_copy(gi, a_sb[:, x0 : x0 + xn], ups[:, :xn])

            if lvl < 6:
                _dbg_finish()
                continue

            # ---- upsample stage 2 + BN + relu, groups of 6 planes -------
            out_sb = big.tile([H, PL * W], BF16, tag="out_sb")
            for gi in range(5):
                t2w = small.tile([HP, 6 * W], BF16, tag="t2w", name="t2w")
                for j3 in range(2):
                    i0 = gi * 6 + 3 * j3
                    t2 = psB.tile([HP, 3 * W], BF16, tag="psb", name="t2")
                    for k3 in range(3):
                        nc.tensor.transpose(
                            t2[:, k3 * W : (k3 + 1) * W],
                            a_sb[:, (i0 + k3) * HP : (i0 + k3 + 1) * HP],
                            id96,
                        )
                    alt_copy(j3, t2w[:, j3 * 3 * W : (j3 + 1) * 3 * W], t2[:])
                for half in range(2):
                    up = psB.tile([H, 3 * W], dt, tag="psb", name="up")
                    nc.tensor.matmul(
                        up[:], mhT, t2w[:, half * 3 * W : (half + 1) * 3 * W]
                    )
                    for j in range(3):
                        ig = gi * 6 + half * 3 + j
                        if ig % 3 != 2:
                            nc.scalar.activation(
                                out_sb[:, ig * W : (ig + 1) * W],
                                up[:, j * W : (j + 1) * W],
                                func=mybir.ActivationFunctionType.Relu,
                                bias=bnb[:, ig : ig + 1],
                                scale=1.0,
                            )
                        else:
                            nc.vector.scalar_tensor_tensor(
                                out_sb[:, ig * W : (ig + 1) * W],
                                up[:, j * W : (j + 1) * W],
                                bnb[:, ig : ig + 1], zeros_sb[:],
                                op0=mybir.AluOpType.add,
                                op1=mybir.AluOpType.max,
                            )
                nc.scalar.dma_start(
                    out3[:, gi * 576 : (gi + 1) * 576],
                    out_sb[:, gi * 576 : (gi + 1) * 576],
                )
          if barrier:
            nc.all_engine_barrier()

    _split_excess_waits(nc)
    return nc


_PROGRAM_CACHE = {}


def _get_program():
    if "nc" not in _PROGRAM_CACHE:
        _PROGRAM_CACHE["nc"] = _build_program()
    return _PROGRAM_CACHE["nc"]


def make_in_maps(p_fea, hu, coord_W, coord_b, query_W, query_b, key_W, key_b,
                 proj_W, bn_gamma, bn_beta, bn_mean, bn_var):
    p_fea = np.asarray(p_fea, np.float32)
    hu = np.asarray(hu, np.float32)
    coord_W = np.asarray(coord_W, np.float32)
    coord_b = np.asarray(coord_b, np.float32)
    query_W = np.asarray(query_W, np.float32)
    query_b = np.asarray(query_b, np.float32)
    key_W = np.asarray(key_W, np.float32)
    key_b = np.asarray(key_b, np.float32)
    proj_W = np.asarray(proj_W, np.float32)
    bn_gamma = np.asarray(bn_gamma, np.float32)
    bn_beta = np.asarray(bn_beta, np.float32)
    bn_mean = np.asarray(bn_mean, np.float32)
    bn_var = np.asarray(bn_var, np.float32)

    # ---- host constant folding ------------------------------------------
    cf9 = np.concatenate(
        [_coord_feats(HP, WP).reshape(8, X), np.ones((1, X), np.float32)],
        axis=0,
    )  # [9, 2304]; cf = A9 @ cf9
    A9 = np.concatenate([coord_W, coord_b[:, None]], axis=1)  # [10, 9]
    K9 = key_W[:, C:] @ A9
    K9[:, 8] += key_b
    Q9 = query_W[:, HID:] @ A9
    Q9[:, 8] += query_b
    Mh = _interp_matrix(H, HP)
    Mw = _interp_matrix(W, WP)
    bn_scale = bn_gamma / np.sqrt(bn_var + EPS)
    bn_bias = bn_beta - bn_mean * bn_scale
    WpS = bn_scale[:, :, None] * proj_W  # [parts, hid, c]
    qW_huT = query_W[:, :HID].T.copy()
    keyW_cT = key_W[:, :C].T.copy()

    stat = np.zeros((C, 70), np.float32)
    stat[:, 0:10] = keyW_cT
    for p in range(PARTS):
        stat[:, 10 + 10 * p : 20 + 10 * p] = WpS[p].T
    statcf = np.zeros((9, 70), np.float32)
    statcf[:, 0:10] = K9.T
    qhu_g = np.zeros((4, 128, 30), np.float32)
    for g in range(4):
        for j in range(PPC):
            qhu_g[g, 32 * g + 10 * j : 32 * g + 10 * j + 10,
                  10 * j : 10 * j + 10] = qW_huT
    qcf = np.zeros((9, 30), np.float32)
    for j in range(PPC):
        qcf[:, 10 * j : 10 * j + 10] = Q9.T

    cbank_bf = np.zeros((128, NCBF), np.float32)
    cbank_bf[:, 0:70] = stat[0:128]
    cbank_bf[:, 70:140] = stat[128:256]
    for g in range(4):
        cbank_bf[:, 140 + 30 * g : 170 + 30 * g] = qhu_g[g]
    cbank_bf[0:9, 260:330] = statcf
    cbank_bf[0:9, 330:360] = qcf
    cbank_bf[0:96, 360:456] = np.eye(96, dtype=np.float32)
    cbank_bf[0:WP, 456:552] = Mw.T
    cbank_bf[0:HP, 552:648] = Mh.T
    cbank_bf = cbank_bf.astype(ml_dtypes.bfloat16)

    cf9_bf = np.ascontiguousarray(cf9).astype(ml_dtypes.bfloat16)

    in_maps = []
    for core in range(8):
        n_idx, s = core // 2, core % 2
        pset = [0, 1, 2] if s == 0 else [3, 4, 5]

        cbank = np.zeros((128, NCB), np.float32)
        for j, p in enumerate(pset):
            cbank[0:H, 10 * j : 10 * j + 10] = bn_bias[p][None, :]
        # mask3_60 / selM60: select this core's parts out of the 6-part KVW^T
        for j in range(PPC):
            p6 = 3 * s + j
            cbank[10 * p6 : 10 * p6 + 10, 30 + 10 * j : 40 + 10 * j] = 1.0
            cbank[10 * p6 : 10 * p6 + 10, 60 + 10 * j : 70 + 10 * j] = np.eye(
                10, dtype=np.float32
            )

        ph = (p_fea[n_idx].reshape(2, 128, X * 4)
              .transpose(1, 0, 2).reshape(128, 2 * X * 4))
        hh = (hu[pset, n_idx].reshape(3, 10, 4, 24, 96)
              .transpose(2, 0, 1, 3, 4).reshape(4, 30, 24 * 96))
        hu4a = np.zeros((4, 32, 24 * 96), np.float32)
        hu4a[:, 0:30] = hh

        in_maps.append({
            "pfe": np.ascontiguousarray(ph).astype(ml_dtypes.bfloat16),
            "hu4": hu4a.reshape(128, 2304).astype(ml_dtypes.bfloat16),
            "cbank": cbank,
            "cbank_bf": cbank_bf,
            "cf9c": cf9_bf,
        })
    return in_maps


def assemble_out(results):
    out = np.empty((PARTS, N, HID, H, W), np.float32)
    for core in range(8):
        n_idx, s = core // 2, core % 2
        pset = [0, 1, 2] if s == 0 else [3, 4, 5]
        r = (results[core]["out3"].astype(np.float32)
             .reshape(H, PPC, HID, W).transpose(1, 2, 0, 3))
        out[pset, n_idx] = r
    return out


def kernel(**inputs):
    in_maps = make_in_maps(**inputs)
    nc = _get_program()
    res = run_bass_kernel_spmd(nc, in_maps, core_ids=list(range(8)))
    return assemble_out(res.results)


# revision 38
# speedup vs baseline: 5.2532x; 1.3928x over previous
"""Trainium2 Bass kernel for nn_Dep_Context_80109730005366.

Math notes (exact restructurings of the reference):
  - ctx = (q @ key) @ value is reassociated as q @ (key @ value); KV is
    [hid, c] so the huge [hw, hw] energy matrix never materializes.
  - The 1x1 conv (proj_W) and the BN scale commute with the bilinear
    upsample, so we contract KV with proj_W into a per-part [hid, hid]
    matrix (KVW) and upsample 10 channels instead of 256.
  - Coord features are input-independent; everything derived from them
    (cf, key/query constant terms) is precomputed on host as tiny matrices.

Sharding: 8 cores = 4 batches x 2 half-part groups. Core k handles batch
n = k//2 and parts {0,1,2} (k even) or {3,4,5} (k odd). Shared per-batch
work (maxpool of p_fea, key, KVW) is duplicated across the 2 cores of a
batch; per-part work is split. (A pair ReduceScatter of partial KVW was
tried to halve the p_fea read, but tiny collectives serialize at ~23us
per op on this part — far more than the 2.4MB of HBM reads they save.)

Bandwidth: p_fea / hu / coord features are uploaded as bf16 (host-side
cast inside kernel()); the stat/q matmuls run bf16 with fp32 PSUM
accumulation and everything downstream of the PSUMs stays fp32. Max rel
error vs the fp32 reference is ~4e-3 (tolerance 2e-2).

Queue discipline: nc.sync carries ONLY input DMAs so that rep k+1's input
stream never queues behind rep k's late output DMAs; outputs go on
nc.scalar. Constants are DMA'd once, outside the rep loop.

Intermediates (keywp, kwT, q, ctxT, a, t2w) and the output are bf16 too;
only the PSUM accumulations and the tiny KVW selection stay fp32.

cbank (fp32): 0:30 bnb [96], 30:60 mask3_60 [60], 60:90 selM60 [60]
cbank_bf (bf16): 0:70 stat0 [128], 70:140 stat1 [128],
  140:260 qhu_g0..3 [128] (query stationaries zero outside row-group g --
  PE operands must sit at base partition 0, so hu uses all 128 partitions
  as 4 row-groups and each group's stationary masks the others),
  260:330 statcf [9], 330:360 qcf [9],
  360:456 id96 [96] (top-left 70x70 doubles as id70),
  456:552 mwT [48], 552:648 mhT [48]
PSUM pools are split by phase: psA/psS/psK serve the front half (stat,
transposes, KVW, q), psB the back half (ctx, upsample) — so rep k+1's
front never waits on rep k's back-half PSUM slots.
"""

import ml_dtypes
import numpy as np

import bass_rust
import concourse.bass as bass
import concourse.tile as tile
from concourse import mybir
from concourse.bass_utils import run_bass_kernel_spmd
from concourse.vector_clock import ScopedClock

EPS = 1e-5
N, C, H, W = 4, 256, 96, 96
HP, WP = 48, 48
HID, PARTS = 10, 6
X = HP * WP        # 2304
PPC = 3            # parts per core
PL = PPC * HID     # planes per core = 30
NCB = 90           # fp32 cbank columns
NCBF = 648         # bf16 cbank columns
F32 = mybir.dt.float32
BF16 = mybir.dt.bfloat16

# ---------------------------------------------------------------------------
# Workaround: this container's walrus codegen rejects instructions carrying
# more than a couple of semaphore waits ("Too many sync wait commands").
# TileContext's exit path puts every outstanding wait on one Drain; spread
# them over a chain of single-wait nops instead.
# ---------------------------------------------------------------------------
_MAX_WAITS = 1


def _patched_drain_and_barrier(self, tick_clock, wait_clock):
    nc = self.nc
    drain_inst = nc.sync.drain()
    wait_clock.add_sem_waits(
        drain_inst.ins, ScopedClock({None: tick_clock.global_clock})
    )
    si = drain_inst.ins.sync_info
    waits = list(si.on_wait) if si is not None else []
    updates = list(si.on_update) if si is not None else []
    if len(waits) > _MAX_WAITS:
        drain_inst.ins.sync_info = bass_rust.SyncInfo(
            on_wait=waits[:_MAX_WAITS], on_update=updates
        )
        rest = waits[_MAX_WAITS:]
        for i in range(0, len(rest), _MAX_WAITS):
            nop = nc.sync.nop(nofuse=True, hint="split_drain_wait")
            nop.ins.sync_info = bass_rust.SyncInfo(
                on_wait=rest[i : i + _MAX_WAITS], on_update=[]
            )
    nc.all_engine_barrier()
    assert self.sems is not None
    popped = nc._tile_sem_poison_stack.pop()
    assert popped is self._sem_poison
    nc.clear_and_free_semaphores(list(self.sems.allocated().values()))
    nc.all_engine_barrier()


tile.TileContext._drain_and_barrier = _patched_drain_and_barrier

_BODY_MAX_WAITS = 1


def _split_excess_waits(nc, maxw=_BODY_MAX_WAITS):
    """Post-pass: any instruction carrying more than `maxw` semaphore waits
    gets the excess hoisted onto same-engine nops inserted right before it
    (the engine sequencer blocks on those first, preserving semantics)."""
    eng_map = {
        mybir.EngineType.SP: nc.sync,
        mybir.EngineType.PE: nc.tensor,
        mybir.EngineType.DVE: nc.vector,
        mybir.EngineType.Activation: nc.scalar,
        mybir.EngineType.Pool: nc.gpsimd,
    }

    def make_nop(engine_type, waits):
        bi = eng_map[engine_type].nop(nofuse=True, hint="wait_split")
        # pop it off the tail of the current bb; we'll splice it manually
        cur = nc.cur_bb.bb
        lst = cur.instructions
        assert lst[-1].name == bi.ins.name
        cur.instructions = lst[:-1]
        bi.ins.sync_info = bass_rust.SyncInfo(on_wait=waits, on_update=[])
        return bi.ins

    for bb in nc.m.functions[0].blocks:
        insts = bb.instructions
        out = []
        changed = False
        for inst in insts:
            si = inst.sync_info
            waits = list(si.on_wait) if si is not None else []
            if len(waits) > maxw:
                updates = list(si.on_update) if si is not None else []
                extra, keep = waits[:-maxw], waits[-maxw:]
                for j in range(0, len(extra), maxw):
                    out.append(make_nop(inst.engine, extra[j : j + maxw]))
                inst.sync_info = bass_rust.SyncInfo(on_wait=keep, on_update=updates)
                changed = True
            out.append(inst)
        if changed:
            bb.instructions = out


# ---------------------------------------------------------------------------
# Host-side constant precomputation (all tiny; heavy tensors stay on device)
# ---------------------------------------------------------------------------
def _coord_feats(hp, wp):
    xs = np.arange(wp, dtype=np.float32)
    ys = np.arange(hp, dtype=np.float32)
    xmin = xs / wp * 2 - 1
    xmax = (xs + 1) / wp * 2 - 1
    xctr = (xmin + xmax) / 2
    ymin = ys / hp * 2 - 1
    ymax = (ys + 1) / hp * 2 - 1
    yctr = (ymin + ymax) / 2
    Xb = lambda v: np.broadcast_to(v[None, :], (hp, wp))
    Yb = lambda v: np.broadcast_to(v[:, None], (hp, wp))
    ones = np.ones((hp, wp), np.float32)
    return np.stack(
        [Xb(xmin), Yb(ymin), Xb(xmax), Yb(ymax), Xb(xctr), Yb(yctr),
         ones / wp, ones / hp], axis=0,
    ).astype(np.float32)


def _interp_matrix(out_n, in_n):
    pos = np.arange(out_n, dtype=np.float32) * ((in_n - 1) / (out_n - 1))
    i0 = np.clip(np.floor(pos).astype(np.int64), 0, in_n - 1)
    i1 = np.clip(i0 + 1, 0, in_n - 1)
    w1 = (pos - i0).astype(np.float32)
    M = np.zeros((out_n, in_n), np.float32)
    for r in range(out_n):
        M[r, i0[r]] += 1 - w1[r]
        M[r, i1[r]] += w1[r]
    return M


# ---------------------------------------------------------------------------
# Device program (built once, shared SPMD across all 8 cores)
# ---------------------------------------------------------------------------
def _build_program(reps=1, barrier=False, use_collective=False, upto="full"):
    nc = bass.Bass()
    dt = F32
    STAGES = ["pool", "stat", "kvw", "q", "ctx", "ups1", "full"]
    lvl = STAGES.index(upto)

    pfe = nc.dram_tensor("pfe", [128, 2 * X * 4], BF16, kind="ExternalInput")
    hu4 = nc.dram_tensor("hu4", [128, 2304], BF16, kind="ExternalInput")
    cbank = nc.dram_tensor("cbank", [128, NCB], dt, kind="ExternalInput")
    cbank_bf = nc.dram_tensor("cbank_bf", [128, NCBF], BF16, kind="ExternalInput")
    cf9c = nc.dram_tensor("cf9c", [9, X], BF16, kind="ExternalInput")
    out3 = nc.dram_tensor("out3", [H, PL * W], BF16, kind="ExternalOutput")

    def alt_copy(idx, out, in_):
        # alternate psum->sbuf copies between ACT and DVE to balance engines
        if idx % 2 == 0:
            nc.scalar.copy(out, in_)
        else:
            nc.vector.tensor_copy(out, in_)

    with tile.TileContext(nc) as tc:
      with tc.tile_pool(name="glob", bufs=1) as glob:
        # ---- constants: loaded once, shared by all reps -----------------
        cb = glob.tile([128, NCB], dt, tag="cbank", name="cbank")
        nc.sync.dma_start(cb[:], cbank[:])
        cbf = glob.tile([128, NCBF], BF16, tag="cbank_bf", name="cbank_bf")
        nc.sync.dma_start(cbf[:], cbank_bf[:])
        cf9_sb = glob.tile([9, X], BF16, tag="cf9", name="cf9")
        nc.sync.dma_start(cf9_sb[:], cf9c[:])
        bnb = cb[0:H, 0:30]
        mask3_60 = cb[0:60, 30:60]
        selM60 = cb[0:60, 60:90]
        stat0 = cbf[0:128, 0:70]
        stat1 = cbf[0:128, 70:140]
        qhu_g = [cbf[0:128, 140 + 30 * g : 170 + 30 * g] for g in range(4)]
        statcf = cbf[0:9, 260:330]
        qcf = cbf[0:9, 330:360]
        id96 = cbf[0:96, 360:456]
        id70 = cbf[0:70, 360:430]
        mwT = cbf[0:WP, 456:552]
        mhT = cbf[0:HP, 552:648]
        zeros_sb = glob.tile([H, W], dt, tag="zeros", name="zeros")
        nc.gpsimd.memset(zeros_sb[:], 0.0)

        for _rep in range(reps):
          with (
            tc.tile_pool(name="pfe_in", bufs=6) as pfe_pool,
            tc.tile_pool(name="p1", bufs=3) as p1_pool,
            tc.tile_pool(name="pf", bufs=3) as pf_pool,
            tc.tile_pool(name="hu", bufs=3) as hu_pool,
            tc.tile_pool(name="big", bufs=3) as big,
            tc.tile_pool(name="small", bufs=3) as small,
            tc.tile_pool(name="psA", bufs=2, space="PSUM") as psA,
            tc.tile_pool(name="psS", bufs=2, space="PSUM") as psS,
            tc.tile_pool(name="psK", bufs=1, space="PSUM") as psK,
            tc.tile_pool(name="psB", bufs=3, space="PSUM") as psB,
          ):
            # ---- input DMAs (all on nc.sync: inputs-only queue) ---------
            pf_t = [
                pf_pool.tile([128, HP, WP], BF16, tag=f"pf{t}", name=f"pf{t}")
                for t in range(2)
            ]
            chunks = []
            for c_i in range(16):
                hh8, t = c_i // 2, c_i % 2
                ch = pfe_pool.tile([128, 12, 96], BF16, tag="chunk", name="chunk")
                nc.sync.dma_start(
                    ch[:],
                    pfe[:, t * 9216 + hh8 * 1152 : t * 9216 + (hh8 + 1) * 1152]
                    .rearrange("c (r w) -> c r w", w=96),
                )
                chunks.append((hh8, t, ch))
                if c_i == 3:
                    hu_sb = hu_pool.tile([128, 24, 96], BF16, tag="hu_sb")
                    nc.sync.dma_start(
                        hu_sb[:], hu4.rearrange("p (r w) -> p r w", w=96)
                    )

            # ---- p_fea maxpool (h-pairs first: dense stage1) ------------
            for hh8, t, ch in chunks:
                p1 = p1_pool.tile([128, 6, 96], BF16, tag="p1", name="p1")
                ch2 = ch.rearrange("c (h2 two) w -> c h2 two w", two=2)
                nc.vector.tensor_max(p1[:], ch2[:, :, 0, :], ch2[:, :, 1, :])
                p1w = p1.rearrange("c h (w2 two) -> c h w2 two", two=2)
                nc.vector.tensor_max(
                    pf_t[t][:, hh8 * 6 : (hh8 + 1) * 6, :],
                    p1w[:, :, :, 0],
                    p1w[:, :, :, 1],
                )

            # ---- hu maxpool (h-pairs first) -----------------------------
            hu1 = hu_pool.tile([128, 12, 96], BF16, tag="hu1")
            hv = hu_sb.rearrange("p (h2 two) w -> p h2 two w", two=2)
            nc.vector.tensor_max(hu1[:], hv[:, :, 0, :], hv[:, :, 1, :])
            hup = hu_pool.tile([128, 12, 48], BF16, tag="hup")
            h1w = hu1.rearrange("p h (w2 two) -> p h w2 two", two=2)
            nc.vector.tensor_max(hup[:], h1w[:, :, :, 0], h1w[:, :, :, 1])
            hup_f = hup.rearrange("p h w -> p (h w)")

            # ---- debug early-exit: zero-fill output and stop ------------
            out_sb0 = None
            if lvl < 6:
                out_sb0 = big.tile([H, PL * W], BF16, tag="out_sb")
                nc.gpsimd.memset(out_sb0[:], 0.0)

            def _dbg_finish():
                nc.scalar.dma_start(out3[:], out_sb0[:])

            if lvl < 1:
                _dbg_finish()
                continue

            # ---- key|WpPf stat matmul over pooled p_fea -----------------
            keywp = big.tile([70, X], BF16, tag="keywp")
            pf_f = [t_.rearrange("c h w -> c (h w)") for t_ in pf_t]
            for xi in range(8):
                x0 = xi * 288
                ps = psA.tile([70, 288], dt, tag="psA", name="ps")
                nc.tensor.matmul(
                    ps[:], stat0, pf_f[0][:, x0 : x0 + 288],
                    start=True, stop=False,
                )
                nc.tensor.matmul(
                    ps[:], stat1, pf_f[1][:, x0 : x0 + 288],
                    start=False, stop=False,
                )
                nc.tensor.matmul(
                    ps[:], statcf, cf9_sb[:, x0 : x0 + 288],
                    start=False, stop=True,
                )
                nc.scalar.copy(keywp[:, x0 : x0 + 288], ps[:])

            if lvl < 2:
                _dbg_finish()
                continue

            # ---- transpose -> KVW^T [60, 10] (all 6 parts) --------------
            kwT = big.tile([128, 18, 70], BF16, tag="kwT")
            for b in range(18):
                tp = psS.tile([128, 70], BF16, tag="pss", name="tp")
                nc.tensor.transpose(
                    tp[:], keywp[:, b * 128 : (b + 1) * 128], id70
                )
                nc.scalar.copy(kwT[:, b, :], tp[:])
            kvwT_ps = psK.tile([60, 10], dt, tag="ck", name="kvwT_ps")
            for b in range(18):
                nc.tensor.matmul(
                    kvwT_ps[:], kwT[:, b, 10:70], kwT[:, b, 0:10],
                    start=(b == 0), stop=(b == 17),
                )
            kvwT_sb = small.tile([60, 10], dt, tag="kvwT_sb")
            nc.vector.tensor_copy(kvwT_sb[:], kvwT_ps[:])

            if lvl < 3:
                _dbg_finish()
                continue

            # ---- q matmuls ----------------------------------------------
            q_sb = big.tile([PL, X], BF16, tag="q_sb")
            for c_ in range(8):
                g, hx = c_ // 2, (c_ % 2) * 288
                x0 = g * 576 + hx
                ps = psA.tile([PL, 288], dt, tag="psA", name="ps")
                nc.tensor.matmul(
                    ps[:], qhu_g[g], hup_f[:, hx : hx + 288],
                    start=True, stop=False,
                )
                nc.tensor.matmul(
                    ps[:], qcf, cf9_sb[:, x0 : x0 + 288], start=False, stop=True
                )
                alt_copy(c_, q_sb[:, x0 : x0 + 288], ps[:])

            if lvl < 4:
                _dbg_finish()
                continue

            # ---- block-diag KVW (this core's 3 parts) -------------------
            S_sb = small.tile([60, PL], dt, tag="S_sb")
            for j in range(3):
                nc.vector.tensor_mul(
                    S_sb[:, j * 10 : (j + 1) * 10], kvwT_sb[:],
                    mask3_60[:, j * 10 : (j + 1) * 10],
                )
            kvwbd_ps = psK.tile([PL, PL], dt, tag="ck", name="kvwbd_ps")
            nc.tensor.matmul(kvwbd_ps[:], S_sb[:], selM60)
            kvwbd = small.tile([PL, PL], BF16, tag="kvwbd")
            nc.scalar.copy(kvwbd[:], kvwbd_ps[:])

            # ---- ctx, transposed, plane-major free layout ---------------
            q3 = q_sb.rearrange("p (h w) -> p h w", h=HP)
            ctxT = big.tile([WP, PL * HP], BF16, tag="ctxT")
            ctxT_hi = ctxT.rearrange("w (i h) -> w h i", i=PL)
            for g3 in range(3):
                cps = psB.tile([WP, 16 * PL], dt, tag="psb", name="cps")
                for hh in range(16):
                    hp_i = g3 * 16 + hh
                    nc.tensor.matmul(
                        cps[:, hh * PL : (hh + 1) * PL],
                        q3[:, hp_i, :],
                        kvwbd[:],
                    )
                cps_v = cps.rearrange("w (h i) -> w h i", h=16)
                nc.scalar.copy(ctxT_hi[:, g3 * 16 : (g3 + 1) * 16, :], cps_v[:])

            if lvl < 5:
                _dbg_finish()
                continue

            # ---- upsample stage 1: contract w' --------------------------
            a_sb = big.tile([W, PL * HP], BF16, tag="a_sb")
            for gi, x0 in enumerate(range(0, PL * HP, 512)):
                xn = min(512, PL * HP - x0)
                ups = psB.tile([W, 512], dt, tag="psb", name="ups")
                nc.tensor.matmul(ups[:, :xn], mwT, ctxT[:, x0 : x0 + xn])
                nc.scalar.copy(a_sb[:, x0 : x0 + xn], ups[:, :xn])

            if lvl < 6:
                _dbg_finish()
                continue

            # ---- upsample stage 2 + BN + relu, groups of 6 planes -------
            out_sb = big.tile([H, PL * W], BF16, tag="out_sb")
            for gi in range(5):
                t2w = small.tile([HP, 6 * W], BF16, tag="t2w", name="t2w")
                for j3 in range(2):
                    i0 = gi * 6 + 3 * j3
                    t2 = psB.tile([HP, 3 * W], BF16, tag="psb", name="t2")
                    for k3 in range(3):
                        nc.tensor.transpose(
                            t2[:, k3 * W : (k3 + 1) * W],
                            a_sb[:, (i0 + k3) * HP : (i0 + k3 + 1) * HP],
                            id96,
                        )
                    nc.scalar.copy(t2w[:, j3 * 3 * W : (j3 + 1) * 3 * W], t2[:])
                for half in range(2):
                    up = psB.tile([H, 3 * W], dt, tag="psb", name="up")
                    nc.tensor.matmul(
                        up[:], mhT, t2w[:, half * 3 * W : (half + 1) * 3 * W]
                    )
                    for j in range(3):
                        ig = gi * 6 + half * 3 + j
                        nc.scalar.activation(
                            out_sb[:, ig * W : (ig + 1) * W],
                            up[:, j * W : (j + 1) * W],
                            func=mybir.ActivationFunctionType.Relu,
                            bias=bnb[:, ig : ig + 1],
                            scale=1.0,
                        )
                nc.scalar.dma_start(
                    out3[:, gi * 576 : (gi + 1) * 576],
                    out_sb[:, gi * 576 : (gi + 1) * 576],
                )
          if barrier:
            nc.all_engine_barrier()

    _split_excess_waits(nc)
    return nc


_PROGRAM_CACHE = {}


def _get_program():
    if "nc" not in _PROGRAM_CACHE:
        _PROGRAM_CACHE["nc"] = _build_program()
    return _PROGRAM_CACHE["nc"]


def make_in_maps(p_fea, hu, coord_W, coord_b, query_W, query_b, key_W, key_b,
                 proj_W, bn_gamma, bn_beta, bn_mean, bn_var):
    p_fea = np.asarray(p_fea, np.float32)
    hu = np.asarray(hu, np.float32)
    coord_W = np.asarray(coord_W, np.float32)
    coord_b = np.asarray(coord_b, np.float32)
    query_W = np.asarray(query_W, np.float32)
    query_b = np.asarray(query_b, np.float32)
    key_W = np.asarray(key_W, np.float32)
    key_b = np.asarray(key_b, np.float32)
    proj_W = np.asarray(proj_W, np.float32)
    bn_gamma = np.asarray(bn_gamma, np.float32)
    bn_beta = np.asarray(bn_beta, np.float32)
    bn_mean = np.asarray(bn_mean, np.float32)
    bn_var = np.asarray(bn_var, np.float32)

    # ---- host constant folding ------------------------------------------
    cf9 = np.concatenate(
        [_coord_feats(HP, WP).reshape(8, X), np.ones((1, X), np.float32)],
        axis=0,
    )  # [9, 2304]; cf = A9 @ cf9
    A9 = np.concatenate([coord_W, coord_b[:, None]], axis=1)  # [10, 9]
    K9 = key_W[:, C:] @ A9
    K9[:, 8] += key_b
    Q9 = query_W[:, HID:] @ A9
    Q9[:, 8] += query_b
    Mh = _interp_matrix(H, HP)
    Mw = _interp_matrix(W, WP)
    bn_scale = bn_gamma / np.sqrt(bn_var + EPS)
    bn_bias = bn_beta - bn_mean * bn_scale
    WpS = bn_scale[:, :, None] * proj_W  # [parts, hid, c]
    qW_huT = query_W[:, :HID].T.copy()
    keyW_cT = key_W[:, :C].T.copy()

    stat = np.zeros((C, 70), np.float32)
    stat[:, 0:10] = keyW_cT
    for p in range(PARTS):
        stat[:, 10 + 10 * p : 20 + 10 * p] = WpS[p].T
    statcf = np.zeros((9, 70), np.float32)
    statcf[:, 0:10] = K9.T
    qhu_g = np.zeros((4, 128, 30), np.float32)
    for g in range(4):
        for j in range(PPC):
            qhu_g[g, 32 * g + 10 * j : 32 * g + 10 * j + 10,
                  10 * j : 10 * j + 10] = qW_huT
    qcf = np.zeros((9, 30), np.float32)
    for j in range(PPC):
        qcf[:, 10 * j : 10 * j + 10] = Q9.T

    cbank_bf = np.zeros((128, NCBF), np.float32)
    cbank_bf[:, 0:70] = stat[0:128]
    cbank_bf[:, 70:140] = stat[128:256]
    for g in range(4):
        cbank_bf[:, 140 + 30 * g : 170 + 30 * g] = qhu_g[g]
    cbank_bf[0:9, 260:330] = statcf
    cbank_bf[0:9, 330:360] = qcf
    cbank_bf[0:96, 360:456] = np.eye(96, dtype=np.float32)
    cbank_bf[0:WP, 456:552] = Mw.T
    cbank_bf[0:HP, 552:648] = Mh.T
    cbank_bf = cbank_bf.astype(ml_dtypes.bfloat16)

    cf9_bf = np.ascontiguousarray(cf9).astype(ml_dtypes.bfloat16)

    in_maps = []
    for core in range(8):
        n_idx, s = core // 2, core % 2
        pset = [0, 1, 2] if s == 0 else [3, 4, 5]

        cbank = np.zeros((128, NCB), np.float32)
        for j, p in enumerate(pset):
            cbank[0:H, 10 * j : 10 * j + 10] = bn_bias[p][None, :]
        # mask3_60 / selM60: select this core's parts out of the 6-part KVW^T
        for j in range(PPC):
            p6 = 3 * s + j
            cbank[10 * p6 : 10 * p6 + 10, 30 + 10 * j : 40 + 10 * j] = 1.0
            cbank[10 * p6 : 10 * p6 + 10, 60 + 10 * j : 70 + 10 * j] = np.eye(
                10, dtype=np.float32
            )

        ph = (p_fea[n_idx].reshape(2, 128, X * 4)
              .transpose(1, 0, 2).reshape(128, 2 * X * 4))
        hh = (hu[pset, n_idx].reshape(3, 10, 4, 24, 96)
              .transpose(2, 0, 1, 3, 4).reshape(4, 30, 24 * 96))
        hu4a = np.zeros((4, 32, 24 * 96), np.float32)
        hu4a[:, 0:30] = hh

        in_maps.append({
            "pfe": np.ascontiguousarray(ph).astype(ml_dtypes.bfloat16),
            "hu4": hu4a.reshape(128, 2304).astype(ml_dtypes.bfloat16),
            "cbank": cbank,
            "cbank_bf": cbank_bf,
            "cf9c": cf9_bf,
        })
    return in_maps


def assemble_out(results):
    out = np.empty((PARTS, N, HID, H, W), np.float32)
    for core in range(8):
        n_idx, s = core // 2, core % 2
        pset = [0, 1, 2] if s == 0 else [3, 4, 5]
        r = (results[core]["out3"].astype(np.float32)
             .reshape(H, PPC, HID, W).transpose(1, 2, 0, 3))
        out[pset, n_idx] = r
    return out


def kernel(**inputs):
    in_maps = make_in_maps(**inputs)
    nc = _get_program()
    res = run_bass_kernel_spmd(nc, in_maps, core_ids=list(range(8)))
    return assemble_out(res.results)
